# revision 67
# baseline (speedup 1.0000x reference)
"""Multi-head attention kernel for Trainium2, SPMD over 8 NeuronCores.

Problem: B=2, S=2048, E=1024, H=16 heads, Dh=64.
  q = per-head q_in @ Wq.T (Wq shared across heads), same for k, v
  attn = softmax(q k^T / 8); ctx = attn @ v; out = concat(ctx) @ Wo.T + bo

Sharding: core c handles batch b=c//4 and heads 4*(c%4)..4*(c%4)+3
(head-parallel attention).  The out projection is sharded by e_out columns
(each core receives 256 rows of Wo, host-sliced), with an AllGather of the
per-head context over the 4 cores of each batch group in between.

All matmuls run in bf16 with fp32 PSUM accumulation; softmax statistics
(row sums / reciprocals) stay fp32.

Layout tricks (avoid transposing activations for the V path):
  scores^T = kin @ (A @ qin^T)     with A = Wq^T Wk (projection fused)
  ctx^T    = Wv @ (vin^T @ P^T)    (vin used in natural layout)
  rowsum   = extra ones-column on vin (rides the PE contraction for free)
"""

import contextlib
import sys

sys.path.insert(0, "/opt/trn_rl_repo")

import numpy as np

import concourse.bass as bass
import concourse.masks as masks
import concourse.tile as tile
from concourse import bacc, mybir
from concourse.bass_utils import run_bass_kernel_spmd

B, S, E, H, Dh = 2, 2048, 1024, 16, 64
N_CORES = 8
HPC = 4          # heads per core
NK = S // 128    # 16 key chunks
EOUT = E // 4    # e_out columns per core

F32 = mybir.dt.float32
BF16 = mybir.dt.bfloat16

_CACHE = {}


def _declare_io(nc):
    io = {}
    io["qin"] = nc.dram_tensor("qin", [S, HPC * Dh], F32, kind="ExternalInput").ap()
    io["kin"] = nc.dram_tensor("kin", [S, HPC * Dh], F32, kind="ExternalInput").ap()
    io["vin"] = nc.dram_tensor("vin", [S, HPC * Dh], F32, kind="ExternalInput").ap()
    io["wq"] = nc.dram_tensor("wq", [Dh, Dh], F32, kind="ExternalInput").ap()
    io["wk"] = nc.dram_tensor("wk", [Dh, Dh], F32, kind="ExternalInput").ap()
    io["wv"] = nc.dram_tensor("wv", [Dh, Dh], F32, kind="ExternalInput").ap()
    io["wo_s"] = nc.dram_tensor("wo_s", [EOUT, E], F32, kind="ExternalInput").ap()
    io["bo_s"] = nc.dram_tensor("bo_s", [2, 128], F32, kind="ExternalInput").ap()
    io["outT"] = nc.dram_tensor("outT", [EOUT, S], F32, kind="ExternalOutput").ap()
    return io


def _body(nc, tc, es, io, it, collective=True):
    """One full MHA iteration. `it` only namespaces pool names."""

    def pool(name, bufs, space="SBUF"):
        return es.enter_context(
            tc.tile_pool(name=f"{name}_{it}", bufs=bufs, space=space)
        )

    qin, kin, vin = io["qin"], io["kin"], io["vin"]
    wq, wk, wv, wo_s, bo_s, outT = (
        io["wq"], io["wk"], io["wv"], io["wo_s"], io["bo_s"], io["outT"],
    )

    stage = pool("stage", 2)          # fp32/bf16 staging for casts
    persist = pool("persist", 1)      # long-lived bf16 tensors
    psum_big = pool("psum_big", 2, space="PSUM")    # [128,1024] = 2 banks x2
    psum_acc = pool("psum_acc", 1, space="PSUM")    # [*, 2048]  = 4 banks x1
    upool = pool("upool", 2)
    ppool = pool("ppool", 6)
    npool1 = pool("npool1", 1)        # rsr / rs_b (rs gets 2 bufs below)
    npool2 = pool("npool2", 2)        # w2n / ctxT
    dram = pool("dram", 1, space="DRAM")

    # identity for PE transposes
    ident = persist.tile([128, 128], F32, tag="ident")
    masks.make_identity(nc, ident[:])

    # ---------------- tiny weight prep ----------------
    wq_sb = persist.tile([Dh, Dh], F32, tag="wq_sb")
    nc.sync.dma_start(out=wq_sb[:], in_=wq[:, :])
    wk_sb = persist.tile([Dh, Dh], F32, tag="wk_sb")
    nc.sync.dma_start(out=wk_sb[:], in_=wk[:, :])
    wq_bf = persist.tile([Dh, Dh], BF16, tag="wq_bf")
    nc.vector.tensor_copy(wq_bf[:], wq_sb[:])
    wk_bf = persist.tile([Dh, Dh], BF16, tag="wk_bf")
    nc.vector.tensor_copy(wk_bf[:], wk_sb[:])

    # A = Wq^T @ Wk   [64,64]
    a_ps = psum_big.tile([Dh, Dh], F32, tag="big")
    nc.tensor.matmul(a_ps[:], wq_bf[:], wk_bf[:], start=True, stop=True)
    a_bf = persist.tile([Dh, Dh], BF16, tag="a_bf")
    nc.vector.tensor_copy(a_bf[:], a_ps[:])

    # WvT = Wv^T via small strided DMA from DRAM (64x64, one-time)
    wvT_sb = persist.tile([Dh, Dh], F32, tag="wvT_sb")
    nc.sync.dma_start(out=wvT_sb[:], in_=wv.rearrange("a b -> b a"))
    wvT_bf = persist.tile([Dh, Dh], BF16, tag="wvT_bf")
    nc.vector.tensor_copy(wvT_bf[:], wvT_sb[:])

    # ---------------- activations: load + PE block-transpose ----------------
    # qT/kT packs: [128, 2048] bf16; pack g holds heads 2g (rows 0-63), 2g+1 (64-127)
    qT = [persist.tile([128, S], BF16, tag=f"qT{g}", name=f"qT{g}") for g in range(2)]
    kT = [persist.tile([128, S], BF16, tag=f"kT{g}", name=f"kT{g}") for g in range(2)]
    # per-head base-partition-0 views; odd heads are DMA-copied after transpose
    hsplit = {}
    for hh in ("q", "k"):
        for j in (1, 3):
            hsplit[(hh, j)] = persist.tile(
                [Dh, S], BF16, tag=f"hsplit_{hh}{j}", name=f"hsplit_{it}_{hh}{j}"
            )
    qTh = [qT[0][0:Dh, :], hsplit[("q", 1)][:], qT[1][0:Dh, :], hsplit[("q", 3)][:]]
    kTh = [kT[0][0:Dh, :], hsplit[("k", 1)][:], kT[1][0:Dh, :], hsplit[("k", 3)][:]]

    # vin_ones: [128, NK, HPC, 65] bf16 (col 64 = 1.0 rides the contraction)
    vin_ones = persist.tile([128, NK, HPC, Dh + 1], BF16, tag="vin_ones")
    nc.vector.memset(vin_ones[:, :, :, Dh : Dh + 1], 1.0)

    qk_stage_tiles = {}

    def emit_stage_qk(src_ap, packs, hh, g, c0=0, c1=NK, hsplit_dma=True):
        """Load chunks [c0,c1) of one [2048,128] half, PE-transpose into the
        pack, and (once the pack is complete) split off the odd head.
        Chunk-ranged so k staging can be split around the first scores."""
        key = (hh, g)
        if key not in qk_stage_tiles:
            # k g=0 is filled by two chunk-ranged calls; pin it to its own
            # buffer so the pool can't recycle it between the calls
            tag, bufs = ("kst0", 1) if key == ("k", 0) else ("astage", 2)
            qk_stage_tiles[key] = stage.tile(
                [128, NK, 128], F32, tag=tag, bufs=bufs, name=f"st_{it}_{hh}{g}"
            )
        st = qk_stage_tiles[key]
        src_r = src_ap[:, 128 * g : 128 * (g + 1)].rearrange("(c p) d -> p c d", p=128)
        for cb in range(c0, c1, 4):  # 4-chunk loads so transposes start early
            ce = min(cb + 4, c1)
            nc.sync.dma_start(out=st[:, cb:ce, :], in_=src_r[:, cb:ce, :])
        for cq in range(c0 // 4, c1 // 4):  # 4 transposes batched per psum bank
            t_ps = psum_big.tile([128, 512], F32, tag="big", name=f"tp_{it}_{hh}{g}_{cq}")
            for ci in range(4):
                c = 4 * cq + ci
                nc.tensor.transpose(
                    t_ps[:, 128 * ci : 128 * (ci + 1)], st[:, c, :], ident[:]
                )
            nc.vector.tensor_copy(packs[g][:, 512 * cq : 512 * (cq + 1)], t_ps[:])
        if hsplit_dma:
            nc.sync.dma_start(
                out=hsplit[(hh, 2 * g + 1)][:], in_=packs[g][Dh : 2 * Dh, :]
            )

    def emit_stage_v(g):
        st = stage.tile([128, NK, 128], F32, tag="astage", name=f"stv_{it}_{g}")
        nc.sync.dma_start(
            out=st[:],
            in_=vin[:, 128 * g : 128 * (g + 1)].rearrange("(c p) d -> p c d", p=128),
        )
        # DVE for both casts: ACT must stay clear for exp (in-order queue:
        # anything enqueued before exp delays it)
        eng = nc.vector.tensor_copy
        eng(
            vin_ones[:, :, 2 * g : 2 * g + 2, 0:Dh],
            st[:].rearrange("p c (jj d) -> p c jj d", jj=2),
        )

    # k chunks 0-7 first (scores m=0.. need them before anything else in the
    # in-order PE queue), then the full q pack (u needs all 16 chunks), then
    # v.  k chunks 8-15 are staged inside head 0 (mid hook at m=2).
    emit_stage_qk(kin, kT, "k", 0, 0, NK // 2, hsplit_dma=False)
    emit_stage_qk(qin, qT, "q", 0)

    def emit_stage_k0b():
        emit_stage_qk(kin, kT, "k", 0, NK // 2, NK)

    def emit_stage_g1():
        emit_stage_qk(qin, qT, "q", 1)
        emit_stage_qk(kin, kT, "k", 1)

    woT = [persist.tile([128, EOUT], BF16, tag=f"woT{c8}", name=f"woT{c8}") for c8 in range(8)]
    bo_sb = persist.tile([128, 2], F32, tag="bo_sb2")

    def emit_wot(r):
        if r == 0:
            for h in range(2):
                nc.sync.dma_start(
                    out=bo_sb[:, h : h + 1],
                    in_=bo_s[h, :].rearrange("(p one) -> p one", one=1),
                )
        w_st = stage.tile([128, E], F32, tag="wostage", bufs=1, name=f"wst_{it}_{r}")
        nc.sync.dma_start(out=w_st[:], in_=wo_s[128 * r : 128 * (r + 1), :])
        for q8 in range(2):  # 4 transposes batched through one psum bank
            t_ps = psum_big.tile([128, 512], F32, tag="big", name=f"wtp_{it}_{r}_{q8}")
            for ci in range(4):
                c8 = 4 * q8 + ci
                nc.tensor.transpose(
                    t_ps[:, 128 * ci : 128 * (ci + 1)], w_st[:, 128 * c8 : 128 * (c8 + 1)], ident[:]
                )
            for ci in range(4):
                c8 = 4 * q8 + ci
                nc.vector.tensor_copy(
                    woT[c8][:, 128 * r : 128 * (r + 1)], t_ps[:, 128 * ci : 128 * (ci + 1)]
                )

    # ---------------- attention per head ----------------
    in_cc = dram.tile([2 * Dh, S], BF16)  # heads 0,1 (AG round 0)
    in_cc2h = [
        dram.tile([2 * Dh, S // 2], BF16, name=f"incc2_{it}_{h}", tag=f"incc2{h}")
        for h in range(2)
    ]  # heads 2,3 staged per q-half, contiguous for the split AG
    ag_outs = [
        dram.tile(
            [512, S], BF16,
            addr_space="Local",
            name=f"agout_{it}_{w}", tag=f"agout{w}",
        )
        for w in range(2)
    ]
    ag2h = [
        dram.tile([512, S // 2], BF16, addr_space="Local",
                  name=f"ag2h_{it}_{h}", tag=f"ag2h{h}")
        for h in range(2)
    ]

    def emit_u(j):
        u_bf = upool.tile([Dh, S], BF16, tag="u", name=f"u_{it}_{j}")
        for t in range(4):
            u_ps = psum_big.tile([Dh, 512], F32, tag="big", name=f"ups_{it}_{j}_{t}")
            nc.tensor.matmul(
                u_ps[:], a_bf[:], qTh[j][:, 512 * t : 512 * (t + 1)],
                start=True, stop=True,
            )
            nc.vector.tensor_copy(u_bf[:, 512 * t : 512 * (t + 1)], u_ps[:])
        return u_bf

    W2_LOOKAHEAD = 3  # score/exp chunk-halves emitted ahead of their W2

    def emit_scores_w2(j, u_bf, mid_emit=None, post_first_exp=None):
        """scores -> exp -> W2 accumulation, then eager psum evacuation.
        W2 emission trails the scores/exp stream by W2_LOOKAHEAD halves so
        the in-order PE queue never head-of-line blocks on psum_acc reuse
        (its evacuation overlaps the next head's first scores).  `mid_emit`
        is a dict {m: callback} fired before chunk m.  Returns (w2_sb, rs)."""
        w2_ps = psum_acc.tile([Dh + 1, S], F32, tag="acc", name=f"w2ps_{it}_{j}")
        pend = []

        def emit_w2(mm, qq, pb):
            for u in range(2):
                nc.tensor.matmul(
                    w2_ps[:, 1024 * qq + 512 * u : 1024 * qq + 512 * (u + 1)],
                    vin_ones[:, mm, j, :],
                    pb[:, 512 * u : 512 * (u + 1)],
                    start=(mm == 0), stop=(mm == NK - 1),
                )

        for m in range(NK):
            if mid_emit is not None and m in mid_emit:
                mid_emit[m]()
            kslice = kTh[j][:, 128 * m : 128 * (m + 1)]
            for qh in range(2):
                sc_ps = psum_big.tile([128, 1024], F32, tag="big", name=f"scps_{it}_{j}_{m}_{qh}")
                for u in range(2):
                    nc.tensor.matmul(
                        sc_ps[:, 512 * u : 512 * (u + 1)],
                        kslice,
                        u_bf[:, 1024 * qh + 512 * u : 1024 * qh + 512 * (u + 1)],
                        start=True, stop=True,
                    )
                p_bf = ppool.tile([128, 1024], BF16, tag="p", name=f"p_{it}_{j}_{m}_{qh}")
                nc.scalar.activation(
                    p_bf[:], sc_ps[:], mybir.ActivationFunctionType.Exp, scale=0.125
                )
                if m == 0 and qh == 0 and post_first_exp is not None:
                    # previous head's rs copies slot in here: ACT would
                    # otherwise idle waiting for this head's next scores
                    post_first_exp()
                pend.append((m, qh, p_bf))
                if len(pend) > W2_LOOKAHEAD:
                    emit_w2(*pend.pop(0))
        while pend:
            emit_w2(*pend.pop(0))
        w2_sb = []
        for qh in range(2):
            w2h = npool2.tile([Dh, S // 2], F32, tag="w2sb", bufs=4, name=f"w2sb_{it}_{j}_{qh}")
            nc.vector.tensor_copy(w2h[:], w2_ps[0:Dh, 1024 * qh : 1024 * (qh + 1)])
            w2_sb.append(w2h[:])

        def emit_rs():
            rs = []
            for qh in range(2):
                rsh = npool1.tile([1, S // 2], F32, tag="rs", bufs=4, name=f"rs_{it}_{j}_{qh}")
                nc.scalar.copy(rsh[:], w2_ps[Dh : Dh + 1, 1024 * qh : 1024 * (qh + 1)])
                rs.append(rsh[:])
            return rs

        return w2_sb, emit_rs

    def emit_ctx_prep(j, qh, rsh):
            rsr = npool1.tile([1, S // 2], F32, tag="rsr", bufs=2, name=f"rsr_{it}_{j}_{qh}")
            nc.vector.reciprocal_approx_fast(out=rsr[:], in_=rsh)
            rs_b = npool1.tile([Dh, S // 2], F32, tag="rs_b", bufs=2, name=f"rsb_{it}_{j}_{qh}")
            nc.gpsimd.partition_broadcast(rs_b[:], rsr[:])
            return rs_b

    def emit_ctx_finish(j, qh, w2h, rs_b):
            w2n_bf = npool2.tile([Dh, S // 2], BF16, tag="w2n", bufs=2, name=f"w2n_{it}_{j}_{qh}")
            nc.vector.tensor_mul(w2n_bf[:], w2h, rs_b[:])
            ctxT_bf = npool2.tile([Dh, S // 2], BF16, tag="ctxT", bufs=2, name=f"ctxT_{it}_{j}_{qh}")
            for t in range(2):
                c_ps = psum_big.tile([Dh, 512], F32, tag="big", name=f"cps_{it}_{j}_{qh}_{t}")
                nc.tensor.matmul(
                    c_ps[:], wvT_bf[:], w2n_bf[:, 512 * t : 512 * (t + 1)],
                    start=True, stop=True,
                )
                nc.vector.tensor_copy(ctxT_bf[:, 512 * t : 512 * (t + 1)], c_ps[:])
            if j < 2:
                nc.sync.dma_start(
                    out=in_cc[Dh * j : Dh * (j + 1), 1024 * qh : 1024 * (qh + 1)],
                    in_=ctxT_bf[:],
                )
            elif qh == 0:
                nc.sync.dma_start(
                    out=in_cc2h[qh][Dh * (j - 2) : Dh * (j - 1), :],
                    in_=ctxT_bf[:],
                )
            else:
                # half 1 goes to the contiguous quarter tiles feeding the
                # final quarter-granular AllGathers
                for t in range(2):
                    nc.sync.dma_start(
                        out=in_cc2q[t][Dh * (j - 2) : Dh * (j - 1), :],
                        in_=ctxT_bf[:, 512 * t : 512 * (t + 1)],
                    )

    def emit_ctx_half(j, qh, w2h, rsh):
        emit_ctx_finish(j, qh, w2h, emit_ctx_prep(j, qh, rsh))

    def emit_ctx(j, w2_sb, rs, after_half=None):
        """normalize + ctx matmuls + staging DMA (deferred one head),
        pipelined in q-halves to keep the serial chain short."""
        for qh in range(2):
            emit_ctx_half(j, qh, w2_sb[qh], rs[qh])
            if after_half is not None:
                after_half(qh)

    # software-pipelined head loop: head j's normalize/ctx is emitted after
    # head j+1's U projection so the in-order PE queue never head-of-line
    # blocks on the (DVE/GPSIMD) normalize chain.  The out projection is
    # split in two rounds around a split AllGather so most of it overlaps
    # the attention phase.
    agch = pool("agch", 1)
    # consolidated spread targets: one tile per AG event -> one spread DMA
    # instead of four (each DMA pays ~1.7us of issue+init latency)
    cch_ev = agch.tile([128, 4, S], BF16, tag="agev", name="agev")
    cch_od = [
        agch.tile([128, 4, S // 2], BF16, tag=f"agod{h}", name=f"agod{h}")
        for h in range(2)
    ]
    opool = pool("opool", 2)
    o_acc = [opool.tile([128, S], F32, tag=f"oacc{h}", bufs=1, name=f"oacc{h}") for h in range(2)]

    def emit_ag(which):
        """AllGather heads (0,1) [which=0] or (2,3) [which=1] of this batch."""
        if collective:
            nc.gpsimd.collective_compute(
                "AllGather",
                mybir.AluOpType.bypass,
                replica_groups=[[0, 1, 2, 3], [4, 5, 6, 7]],
                ins=[in_cc[:, :].opt()],
                outs=[ag_outs[which].opt()],
            )
        else:
            # sim stand-in: a light dep edge; real AG runs on TOPSP silicon
            nc.sync.dma_start(out=ag_outs[which][0:128, :], in_=in_cc[:, :])
        # chunk c8 = heads {2c8, 2c8+1}; AG round `which` supplies parity-
        # matching chunks: ag_outs[w] slab r = heads {4r+2w, 4r+2w+1} = chunk 2r+w
        assert which == 0
        nc.sync.dma_start(
            out=cch_ev[:], in_=ag_outs[which].rearrange("(r p) q -> p r q", p=128)
        )

    def emit_ag2(h):
        """AllGather heads (2,3), q-column half h only, so the tail pipelines."""
        if collective:
            nc.gpsimd.collective_compute(
                "AllGather",
                mybir.AluOpType.bypass,
                replica_groups=[[0, 1, 2, 3], [4, 5, 6, 7]],
                ins=[in_cc2h[h][:, :].opt()],
                outs=[ag2h[h].opt()],
            )
        else:
            nc.sync.dma_start(out=ag2h[h][0:128, :], in_=in_cc2h[h][:, :])
        nc.sync.dma_start(
            out=cch_od[h][:], in_=ag2h[h].rearrange("(r p) q -> p r q", p=128)
        )

    # contiguous quarter staging for the very last AG (collective inputs
    # must be contiguous, so column views of in_cc2h can't be used)
    in_cc2q = [
        dram.tile([2 * Dh, 512], BF16, name=f"incc2q_{it}_{t}", tag=f"incc2q{t}")
        for t in range(2)
    ]
    ag2q = [
        dram.tile([512, 512], BF16, addr_space="Local",
                  name=f"ag2q_{it}_{t}", tag=f"ag2q{t}")
        for t in range(2)
    ]

    def emit_ag2q(t):
        """AllGather heads (2,3), q-column quarter t of half 1."""
        if collective:
            nc.gpsimd.collective_compute(
                "AllGather",
                mybir.AluOpType.bypass,
                replica_groups=[[0, 1, 2, 3], [4, 5, 6, 7]],
                ins=[in_cc2q[t][:, :].opt()],
                outs=[ag2q[t].opt()],
            )
        else:
            nc.scalar.dma_start(out=ag2q[t][0:128, :], in_=in_cc2q[t][:, :])
        # spread issued from the (idle-at-tail) Pool queue: cheap issue slot
        # and no head-of-line blocking behind SP's earlier DMAs
        nc.scalar.dma_start(
            out=cch_od[1][:, :, 512 * t : 512 * (t + 1)],
            in_=ag2q[t].rearrange("(r p) q -> p r q", p=128),
        )

    def emit_oproj_group(round_, sh, h, acc_eng=None):
                o_ps = psum_big.tile([128, 1024], F32, tag="big", name=f"ops_{it}_{round_}_{h}_{sh}")
                for i, r in enumerate(range(4)):
                    c8 = 2 * r + round_
                    for u in range(2):
                        rhs = (
                            cch_ev[:, r, 1024 * sh + 512 * u : 1024 * sh + 512 * (u + 1)]
                            if round_ == 0
                            else cch_od[sh][:, r, 512 * u : 512 * (u + 1)]
                        )
                        nc.tensor.matmul(
                            o_ps[:, 512 * u : 512 * (u + 1)],
                            woT[c8][:, 128 * h : 128 * (h + 1)],
                            rhs,
                            start=(i == 0), stop=(i == 3),
                        )
                if round_ == 0:
                    (acc_eng or nc.vector.tensor_copy)(
                        o_acc[h][:, 1024 * sh : 1024 * (sh + 1)], o_ps[:]
                    )
                else:
                    o_sb = opool.tile([128, 1024], F32, tag="osb", name=f"osb_{it}_{h}_{sh}")
                    eng = nc.vector
                    eng.scalar_tensor_tensor(
                        o_sb[:], o_ps[:], bo_sb[:, h : h + 1],
                        o_acc[h][:, 1024 * sh : 1024 * (sh + 1)],
                        mybir.AluOpType.add, mybir.AluOpType.add,
                    )
                    nc.sync.dma_start(
                        out=outT[128 * h : 128 * (h + 1), 1024 * sh : 1024 * (sh + 1)],
                        in_=o_sb[:],
                    )

    def emit_oproj(round_):
        """Accumulate 4 chunks (parity `round_`) into o_acc (round 0) or
        finish with bias into outT (round 1)."""
        for sh in range(2):
            for h in range(2):
                emit_oproj_group(round_, sh, h)

    def emit_last_head(u_bf, prev_ctx, prev_rs):
        """Head 3 with q-half-outer loops: half 0's normalize/ctx/AG overlap
        half 1's attention, shrinking the serial tail."""
        j = HPC - 1
        w2_ps = psum_acc.tile([Dh + 1, S], F32, tag="acc", name=f"w2ps_{it}_last")
        halves = {}

        def attn_span(s0, s1, mids=None, post_first_exp=None, tail_emits=None):
            """Attention over 512-col q slices [s0, s1) of this head."""
            pend = []
            ns = s1 - s0

            def emit_w2(mm, pb):
                for u in range(ns):
                    nc.tensor.matmul(
                        w2_ps[:, 512 * (s0 + u) : 512 * (s0 + u + 1)],
                        vin_ones[:, mm, j, :],
                        pb[:, 512 * u : 512 * (u + 1)],
                        start=(mm == 0), stop=(mm == NK - 1),
                    )

            for m in range(NK):
                if mids is not None and m in mids:
                    mids[m]()
                kslice = kTh[j][:, 128 * m : 128 * (m + 1)]
                sc_ps = psum_big.tile([128, 512 * ns], F32, tag="big", name=f"scpsL_{s0}_{m}")
                for u in range(ns):
                    nc.tensor.matmul(
                        sc_ps[:, 512 * u : 512 * (u + 1)],
                        kslice,
                        u_bf[:, 512 * (s0 + u) : 512 * (s0 + u + 1)],
                        start=True, stop=True,
                    )
                p_bf = ppool.tile([128, 512 * ns], BF16, tag="p", name=f"pL_{s0}_{m}")
                nc.scalar.activation(
                    p_bf[:], sc_ps[:], mybir.ActivationFunctionType.Exp, scale=0.125
                )
                if m == 0 and post_first_exp is not None:
                    post_first_exp()
                pend.append((m, p_bf))
                if len(pend) > W2_LOOKAHEAD:
                    emit_w2(*pend.pop(0))
            while pend:
                emit_w2(*pend.pop(0))
                # interleaved tail work keeps PE hot (full p-state) through
                # the exp-bound last chunks
                if tail_emits:
                    tail_emits.pop(0)()

        def evac_half(qh):
            w2h = npool2.tile([Dh, S // 2], F32, tag="w2sb", bufs=4, name=f"w2sbL_{qh}")
            nc.vector.tensor_copy(w2h[:], w2_ps[0:Dh, 1024 * qh : 1024 * (qh + 1)])
            rsh = npool1.tile([1, S // 2], F32, tag="rs", bufs=4, name=f"rsL_{qh}")
            nc.scalar.copy(rsh[:], w2_ps[Dh : Dh + 1, 1024 * qh : 1024 * (qh + 1)])
            halves[qh] = (w2h[:], rsh[:])

        def tail_quarter_norm(t):
            """evac + normalize of q-column quarter t of half 1."""
            w2q = npool2.tile([Dh, 512], F32, tag="w2sb", bufs=4, name=f"w2qL_{t}")
            nc.vector.tensor_copy(w2q[:], w2_ps[0:Dh, 1024 + 512 * t : 1024 + 512 * (t + 1)])
            rsq = npool1.tile([1, 512], F32, tag="rs", bufs=4, name=f"rsqL_{t}")
            nc.scalar.copy(rsq[:], w2_ps[Dh : Dh + 1, 1024 + 512 * t : 1024 + 512 * (t + 1)])
            rsrq = npool1.tile([1, 512], F32, tag="rsr", bufs=2, name=f"rsrqL_{t}")
            nc.vector.reciprocal_approx_fast(out=rsrq[:], in_=rsq[:])
            rsbq = npool1.tile([Dh, 512], F32, tag="rs_b", bufs=2, name=f"rsbqL_{t}")
            nc.gpsimd.partition_broadcast(rsbq[:], rsrq[:])
            w2nq = npool2.tile([Dh, 512], BF16, tag="w2n", bufs=2, name=f"w2nqL_{t}")
            nc.vector.tensor_mul(w2nq[:], w2q[:], rsbq[:])
            return w2nq

        def tail_quarter_ctx(t, w2nq):
            c_ps = psum_big.tile([Dh, 512], F32, tag="big", name=f"cpsqL_{t}")
            nc.tensor.matmul(c_ps[:], wvT_bf[:], w2nq[:], start=True, stop=True)
            ctxq = npool2.tile([Dh, 512], BF16, tag="ctxT", bufs=2, name=f"ctxqL_{t}")
            nc.vector.tensor_copy(ctxq[:], c_ps[:])
            nc.scalar.dma_start(out=in_cc2q[t][Dh : 2 * Dh, :], in_=ctxq[:])
            emit_ag2q(t)

        def tail_quarter_oproj(t):
            for h in range(2):
                o_ps = psum_big.tile([128, 512], F32, tag="big", name=f"opsq_{t}_{h}")
                for r in range(4):
                    nc.tensor.matmul(
                        o_ps[:],
                        woT[2 * r + 1][:, 128 * h : 128 * (h + 1)],
                        cch_od[1][:, r, 512 * t : 512 * (t + 1)],
                        start=(r == 0), stop=(r == 3),
                    )
                o_sb = opool.tile([128, 512], F32, tag="osb", name=f"osbq_{t}_{h}")
                nc.vector.scalar_tensor_tensor(
                    o_sb[:], o_ps[:], bo_sb[:, h : h + 1],
                    o_acc[h][:, 1024 + 512 * t : 1024 + 512 * (t + 1)],
                    mybir.AluOpType.add, mybir.AluOpType.add,
                )
                nc.scalar.dma_start(
                    out=outT[128 * h : 128 * (h + 1), 1024 + 512 * t : 1024 + 512 * (t + 1)],
                    in_=o_sb[:],
                )

        # oproj round 0 rides the half-span (2x ACT runway per chunk absorbs
        # the 1.7us PE detours); the narrow quarter spans get only tiny mids
        attn_span(
            0, 2,
            mids={
                4: prev_ctx,
                8: lambda: emit_oproj_group(0, 0, 0),
                10: lambda: emit_oproj_group(0, 0, 1),
                12: lambda: emit_oproj_group(0, 1, 0),
                14: lambda: emit_oproj_group(0, 1, 1),
            },
            post_first_exp=prev_rs,
        )
        evac_half(0)
        # quarter 2 (q cols 1024:1536): its post-processing hides under
        # quarter 3's attention, so only ONE DMA->AG ladder is ever exposed
        attn_span(
            2, 3,
            mids={2: lambda: (emit_ctx_half(j, 0, *halves[0]), emit_ag2(0))},
            tail_emits=[lambda: emit_oproj_group(1, 0, 0)],
        )
        wn0 = tail_quarter_norm(0)
        # quarter 3 (q cols 1536:2048): quarter 2's ctx/AG launches at m=1;
        # its oproj lands in this flush once the spread arrives
        attn_span(
            3, 4,
            mids={1: lambda: tail_quarter_ctx(0, wn0)},
            tail_emits=[
                lambda: emit_oproj_group(1, 0, 1),
                lambda: tail_quarter_oproj(0),
            ],
        )
        wn1 = tail_quarter_norm(1)
        tail_quarter_ctx(1, wn1)
        tail_quarter_oproj(1)

    u_next = [emit_u(0)]
    # v staging emitted after u(0) so its Pool-queue cast doesn't head-of-line
    # block u's psum evacuations (which gate the first scores)
    emit_stage_v(0)
    pending = None      # (j, w2_sb) of the previous head
    rs_store = {}       # j -> rs APs (filled by the deferred emitters)
    rs_emitters = {}    # j -> closure that emits the rs copies
    for j in range(HPC - 1):
        def mid():
            if j == 1:
                emit_stage_v(1)
                emit_wot(0)
            if j == 2:
                emit_wot(1)
            u_next.append(emit_u(j + 1))
        mids = {NK // 2: mid}
        if j == 0:
            mids[2] = emit_stage_k0b
        if j == 1:
            # g1 staging spread over head 1 (first needed by u(2) at m=8 /
            # head-2 scores) instead of one big detour inside head 0
            mids[2] = lambda: emit_stage_qk(qin, qT, "q", 1)
            mids[5] = lambda: emit_stage_qk(kin, kT, "k", 1)

        def post_fe(jp=j - 1):
            if jp in rs_emitters:
                rs_store[jp] = rs_emitters.pop(jp)()

        u_cur = u_next[-1]
        w2_sb, emit_rs = emit_scores_w2(
            j, u_cur, mid_emit=mids, post_first_exp=post_fe if j > 0 else None
        )
        rs_emitters[j] = emit_rs
        if pending is not None:
            jprev, w2_prev = pending
            emit_ctx(jprev, w2_prev, rs_store[jprev])
            if jprev == 1:
                emit_ag(0)
        pending = (j, w2_sb)
    jprev, w2_prev = pending
    emit_last_head(
        u_next[-1],
        prev_ctx=lambda: emit_ctx(jprev, w2_prev, rs_store[jprev]),
        prev_rs=lambda: rs_store.update({jprev: rs_emitters.pop(jprev)()}),
    )


def _build(repeats=1, collective=True):
    key = (repeats, collective)
    if key in _CACHE:
        return _CACHE[key]
    ndev = N_CORES if collective else 1
    nc = bacc.Bacc("TRN2", target_bir_lowering=False, debug=False, num_devices=ndev)
    io = _declare_io(nc)
    with tile.TileContext(nc) as tc:
        for it in range(repeats):
            with contextlib.ExitStack() as es:
                _body(nc, tc, es, io, it, collective=collective)
    nc.compile()
    _CACHE[key] = nc
    return nc


def kernel(k_in, q_in, v_in, Wq, Wk, Wv, Wo, bo, _repeats=1, _results_hook=None):
    k_in = np.asarray(k_in, dtype=np.float32)
    q_in = np.asarray(q_in, dtype=np.float32)
    v_in = np.asarray(v_in, dtype=np.float32)
    Wq = np.ascontiguousarray(np.asarray(Wq, dtype=np.float32))
    Wk = np.ascontiguousarray(np.asarray(Wk, dtype=np.float32))
    Wv = np.ascontiguousarray(np.asarray(Wv, dtype=np.float32))
    Wo = np.asarray(Wo, dtype=np.float32)
    bo = np.asarray(bo, dtype=np.float32)

    nc = _build(_repeats)

    in_maps = []
    for c in range(N_CORES):
        b, q4 = c // 4, c % 4
        sl = slice(256 * q4, 256 * (q4 + 1))
        in_maps.append(
            {
                "qin": np.ascontiguousarray(q_in[b, :, sl]),
                "kin": np.ascontiguousarray(k_in[b, :, sl]),
                "vin": np.ascontiguousarray(v_in[b, :, sl]),
                "wq": Wq,
                "wk": Wk,
                "wv": Wv,
                "wo_s": np.ascontiguousarray(Wo[sl, :]),
                "bo_s": np.ascontiguousarray(bo[sl].reshape(2, 128)),
            }
        )

    res = run_bass_kernel_spmd(nc, in_maps, core_ids=list(range(N_CORES)))
    if _results_hook is not None:
        _results_hook(res)

    out = np.empty((B, S, E), dtype=np.float32)
    for c in range(N_CORES):
        b, q4 = c // 4, c % 4
        out[b, :, 256 * q4 : 256 * (q4 + 1)] = res.results[c]["outT"].T
    return out



# revision 68
# speedup vs baseline: 1.0649x; 1.0649x over previous
"""Multi-head attention kernel for Trainium2, SPMD over 8 NeuronCores.

Problem: B=2, S=2048, E=1024, H=16 heads, Dh=64.
  q = per-head q_in @ Wq.T (Wq shared across heads), same for k, v
  attn = softmax(q k^T / 8); ctx = attn @ v; out = concat(ctx) @ Wo.T + bo

Sharding: core c handles batch b=c//4 and heads 4*(c%4)..4*(c%4)+3
(head-parallel attention).  The out projection is sharded by e_out columns
(each core receives 256 rows of Wo, host-sliced), with an AllGather of the
per-head context over the 4 cores of each batch group in between.

All matmuls run in bf16 with fp32 PSUM accumulation; softmax statistics
(row sums / reciprocals) stay fp32.

Layout tricks (avoid transposing activations for the V path):
  scores^T = kin @ (A @ qin^T)     with A = Wq^T Wk (projection fused)
  ctx^T    = Wv @ (vin^T @ P^T)    (vin used in natural layout)
  rowsum   = extra ones-column on vin (rides the PE contraction for free)
"""

import contextlib
import sys

sys.path.insert(0, "/opt/trn_rl_repo")

import numpy as np

import concourse.bass as bass
import concourse.masks as masks
import concourse.tile as tile
from concourse import bacc, mybir
from concourse.bass_utils import run_bass_kernel_spmd

B, S, E, H, Dh = 2, 2048, 1024, 16, 64
N_CORES = 8
HPC = 4          # heads per core
NK = S // 128    # 16 key chunks
EOUT = E // 4    # e_out columns per core

F32 = mybir.dt.float32
BF16 = mybir.dt.bfloat16

_CACHE = {}


def _declare_io(nc):
    io = {}
    io["qin"] = nc.dram_tensor("qin", [S, HPC * Dh], F32, kind="ExternalInput").ap()
    io["kin"] = nc.dram_tensor("kin", [S, HPC * Dh], F32, kind="ExternalInput").ap()
    io["vin"] = nc.dram_tensor("vin", [S, HPC * Dh], F32, kind="ExternalInput").ap()
    io["wq"] = nc.dram_tensor("wq", [Dh, Dh], F32, kind="ExternalInput").ap()
    io["wk"] = nc.dram_tensor("wk", [Dh, Dh], F32, kind="ExternalInput").ap()
    io["wv"] = nc.dram_tensor("wv", [Dh, Dh], F32, kind="ExternalInput").ap()
    io["wo_s"] = nc.dram_tensor("wo_s", [EOUT, E], F32, kind="ExternalInput").ap()
    io["bo_s"] = nc.dram_tensor("bo_s", [2, 128], F32, kind="ExternalInput").ap()
    io["outT"] = nc.dram_tensor("outT", [EOUT, S], F32, kind="ExternalOutput").ap()
    return io


def _body(nc, tc, es, io, it, collective=True):
    """One full MHA iteration. `it` only namespaces pool names."""

    def pool(name, bufs, space="SBUF"):
        return es.enter_context(
            tc.tile_pool(name=f"{name}_{it}", bufs=bufs, space=space)
        )

    qin, kin, vin = io["qin"], io["kin"], io["vin"]
    wq, wk, wv, wo_s, bo_s, outT = (
        io["wq"], io["wk"], io["wv"], io["wo_s"], io["bo_s"], io["outT"],
    )

    stage = pool("stage", 2)          # fp32/bf16 staging for casts
    persist = pool("persist", 1)      # long-lived bf16 tensors
    psum_big = pool("psum_big", 2, space="PSUM")    # [128,1024] = 2 banks x2
    psum_acc = pool("psum_acc", 1, space="PSUM")    # [*, 2048]  = 4 banks x1
    upool = pool("upool", 2)
    ppool = pool("ppool", 6)
    npool1 = pool("npool1", 1)        # rsr / rs_b (rs gets 2 bufs below)
    npool2 = pool("npool2", 2)        # w2n / ctxT
    dram = pool("dram", 1, space="DRAM")

    # identity for PE transposes
    ident = persist.tile([128, 128], F32, tag="ident")
    masks.make_identity(nc, ident[:])

    # ---------------- tiny weight prep ----------------
    wq_sb = persist.tile([Dh, Dh], F32, tag="wq_sb")
    nc.sync.dma_start(out=wq_sb[:], in_=wq[:, :])
    wk_sb = persist.tile([Dh, Dh], F32, tag="wk_sb")
    nc.sync.dma_start(out=wk_sb[:], in_=wk[:, :])
    wq_bf = persist.tile([Dh, Dh], BF16, tag="wq_bf")
    nc.vector.tensor_copy(wq_bf[:], wq_sb[:])
    wk_bf = persist.tile([Dh, Dh], BF16, tag="wk_bf")
    nc.vector.tensor_copy(wk_bf[:], wk_sb[:])

    # A = Wq^T @ Wk   [64,64]
    a_ps = psum_big.tile([Dh, Dh], F32, tag="big")
    nc.tensor.matmul(a_ps[:], wq_bf[:], wk_bf[:], start=True, stop=True)
    a_bf = persist.tile([Dh, Dh], BF16, tag="a_bf")
    nc.vector.tensor_copy(a_bf[:], a_ps[:])

    # WvT = Wv^T via small strided DMA from DRAM (64x64, one-time)
    wvT_sb = persist.tile([Dh, Dh], F32, tag="wvT_sb")
    nc.sync.dma_start(out=wvT_sb[:], in_=wv.rearrange("a b -> b a"))
    wvT_bf = persist.tile([Dh, Dh], BF16, tag="wvT_bf")
    nc.vector.tensor_copy(wvT_bf[:], wvT_sb[:])

    # ---------------- activations: load + PE block-transpose ----------------
    # qT/kT packs: [128, 2048] bf16; pack g holds heads 2g (rows 0-63), 2g+1 (64-127)
    qT = [persist.tile([128, S], BF16, tag=f"qT{g}", name=f"qT{g}") for g in range(2)]
    kT = [persist.tile([128, S], BF16, tag=f"kT{g}", name=f"kT{g}") for g in range(2)]
    # per-head base-partition-0 views; odd heads are DMA-copied after transpose
    hsplit = {}
    for hh in ("q", "k"):
        for j in (1, 3):
            hsplit[(hh, j)] = persist.tile(
                [Dh, S], BF16, tag=f"hsplit_{hh}{j}", name=f"hsplit_{it}_{hh}{j}"
            )
    qTh = [qT[0][0:Dh, :], hsplit[("q", 1)][:], qT[1][0:Dh, :], hsplit[("q", 3)][:]]
    kTh = [kT[0][0:Dh, :], hsplit[("k", 1)][:], kT[1][0:Dh, :], hsplit[("k", 3)][:]]

    # vin_ones: [128, NK, HPC, 65] bf16 (col 64 = 1.0 rides the contraction)
    vin_ones = persist.tile([128, NK, HPC, Dh + 1], BF16, tag="vin_ones")
    nc.vector.memset(vin_ones[:, :, :, Dh : Dh + 1], 1.0)

    qk_stage_tiles = {}

    def emit_stage_qk(src_ap, packs, hh, g, c0=0, c1=NK, hsplit_dma=True):
        """Load chunks [c0,c1) of one [2048,128] half, PE-transpose into the
        pack, and (once the pack is complete) split off the odd head.
        Chunk-ranged so k staging can be split around the first scores."""
        key = (hh, g)
        if key not in qk_stage_tiles:
            # k g=0 is filled by two chunk-ranged calls; pin it to its own
            # buffer so the pool can't recycle it between the calls
            tag, bufs = ("kst0", 1) if key == ("k", 0) else ("astage", 2)
            qk_stage_tiles[key] = stage.tile(
                [128, NK, 128], F32, tag=tag, bufs=bufs, name=f"st_{it}_{hh}{g}"
            )
        st = qk_stage_tiles[key]
        src_r = src_ap[:, 128 * g : 128 * (g + 1)].rearrange("(c p) d -> p c d", p=128)
        for cb in range(c0, c1, 4):  # 4-chunk loads so transposes start early
            ce = min(cb + 4, c1)
            nc.sync.dma_start(out=st[:, cb:ce, :], in_=src_r[:, cb:ce, :])
        for cq in range(c0 // 4, c1 // 4):  # 4 transposes batched per psum bank
            t_ps = psum_big.tile([128, 512], F32, tag="big", name=f"tp_{it}_{hh}{g}_{cq}")
            for ci in range(4):
                c = 4 * cq + ci
                nc.tensor.transpose(
                    t_ps[:, 128 * ci : 128 * (ci + 1)], st[:, c, :], ident[:]
                )
            nc.vector.tensor_copy(packs[g][:, 512 * cq : 512 * (cq + 1)], t_ps[:])
        if hsplit_dma:
            nc.sync.dma_start(
                out=hsplit[(hh, 2 * g + 1)][:], in_=packs[g][Dh : 2 * Dh, :]
            )

    def emit_stage_v(g):
        st = stage.tile([128, NK, 128], F32, tag="astage", name=f"stv_{it}_{g}")
        nc.sync.dma_start(
            out=st[:],
            in_=vin[:, 128 * g : 128 * (g + 1)].rearrange("(c p) d -> p c d", p=128),
        )
        # DVE for both casts: ACT must stay clear for exp (in-order queue:
        # anything enqueued before exp delays it)
        eng = nc.vector.tensor_copy
        eng(
            vin_ones[:, :, 2 * g : 2 * g + 2, 0:Dh],
            st[:].rearrange("p c (jj d) -> p c jj d", jj=2),
        )

    # k chunks 0-7 first (scores m=0.. need them before anything else in the
    # in-order PE queue), then the full q pack (u needs all 16 chunks), then
    # v.  k chunks 8-15 are staged inside head 0 (mid hook at m=2).
    emit_stage_qk(kin, kT, "k", 0, 0, NK // 2, hsplit_dma=False)
    emit_stage_qk(qin, qT, "q", 0)

    def emit_stage_k0b():
        emit_stage_qk(kin, kT, "k", 0, NK // 2, NK)

    def emit_stage_g1():
        emit_stage_qk(qin, qT, "q", 1)
        emit_stage_qk(kin, kT, "k", 1)

    woT = [persist.tile([128, EOUT], BF16, tag=f"woT{c8}", name=f"woT{c8}") for c8 in range(8)]
    bo_sb = persist.tile([128, 2], F32, tag="bo_sb2")

    def emit_wot(r):
        if r == 0:
            for h in range(2):
                nc.sync.dma_start(
                    out=bo_sb[:, h : h + 1],
                    in_=bo_s[h, :].rearrange("(p one) -> p one", one=1),
                )
        w_st = stage.tile([128, E], F32, tag="wostage", bufs=1, name=f"wst_{it}_{r}")
        nc.sync.dma_start(out=w_st[:], in_=wo_s[128 * r : 128 * (r + 1), :])
        for q8 in range(2):  # 4 transposes batched through one psum bank
            t_ps = psum_big.tile([128, 512], F32, tag="big", name=f"wtp_{it}_{r}_{q8}")
            for ci in range(4):
                c8 = 4 * q8 + ci
                nc.tensor.transpose(
                    t_ps[:, 128 * ci : 128 * (ci + 1)], w_st[:, 128 * c8 : 128 * (c8 + 1)], ident[:]
                )
            for ci in range(4):
                c8 = 4 * q8 + ci
                nc.vector.tensor_copy(
                    woT[c8][:, 128 * r : 128 * (r + 1)], t_ps[:, 128 * ci : 128 * (ci + 1)]
                )

    # ---------------- attention per head ----------------
    in_cc = dram.tile([2 * Dh, S], BF16)  # heads 0,1 (AG round 0)
    in_cc2h = [
        dram.tile([2 * Dh, S // 2], BF16, name=f"incc2_{it}_{h}", tag=f"incc2{h}")
        for h in range(2)
    ]  # heads 2,3 staged per q-half, contiguous for the split AG
    ag_outs = [
        dram.tile(
            [512, S], BF16,
            addr_space="Local",
            name=f"agout_{it}_{w}", tag=f"agout{w}",
        )
        for w in range(2)
    ]
    ag2h = [
        dram.tile([512, S // 2], BF16, addr_space="Local",
                  name=f"ag2h_{it}_{h}", tag=f"ag2h{h}")
        for h in range(2)
    ]

    def emit_u(j):
        u_bf = upool.tile([Dh, S], BF16, tag="u", name=f"u_{it}_{j}")
        for t in range(4):
            u_ps = psum_big.tile([Dh, 512], F32, tag="big", name=f"ups_{it}_{j}_{t}")
            nc.tensor.matmul(
                u_ps[:], a_bf[:], qTh[j][:, 512 * t : 512 * (t + 1)],
                start=True, stop=True,
            )
            nc.vector.tensor_copy(u_bf[:, 512 * t : 512 * (t + 1)], u_ps[:])
        return u_bf

    W2_LOOKAHEAD = 3  # score/exp chunk-halves emitted ahead of their W2

    def emit_scores_w2(j, u_bf, mid_emit=None, post_first_exp=None):
        """scores -> exp -> W2 accumulation, then eager psum evacuation.
        W2 emission trails the scores/exp stream by W2_LOOKAHEAD halves so
        the in-order PE queue never head-of-line blocks on psum_acc reuse
        (its evacuation overlaps the next head's first scores).  `mid_emit`
        is a dict {m: callback} fired before chunk m.  Returns (w2_sb, rs)."""
        w2_ps = psum_acc.tile([Dh + 1, S], F32, tag="acc", name=f"w2ps_{it}_{j}")
        pend = []

        def emit_w2(mm, qq, pb):
            for u in range(2):
                nc.tensor.matmul(
                    w2_ps[:, 1024 * qq + 512 * u : 1024 * qq + 512 * (u + 1)],
                    vin_ones[:, mm, j, :],
                    pb[:, 512 * u : 512 * (u + 1)],
                    start=(mm == 0), stop=(mm == NK - 1),
                )

        for m in range(NK):
            if mid_emit is not None and m in mid_emit:
                mid_emit[m]()
            kslice = kTh[j][:, 128 * m : 128 * (m + 1)]
            for qh in range(2):
                sc_ps = psum_big.tile([128, 1024], F32, tag="big", name=f"scps_{it}_{j}_{m}_{qh}")
                for u in range(2):
                    nc.tensor.matmul(
                        sc_ps[:, 512 * u : 512 * (u + 1)],
                        kslice,
                        u_bf[:, 1024 * qh + 512 * u : 1024 * qh + 512 * (u + 1)],
                        start=True, stop=True,
                    )
                p_bf = ppool.tile([128, 1024], BF16, tag="p", name=f"p_{it}_{j}_{m}_{qh}")
                nc.scalar.activation(
                    p_bf[:], sc_ps[:], mybir.ActivationFunctionType.Exp, scale=0.125
                )
                if m == 0 and qh == 0 and post_first_exp is not None:
                    # previous head's rs copies slot in here: ACT would
                    # otherwise idle waiting for this head's next scores
                    post_first_exp()
                pend.append((m, qh, p_bf))
                if len(pend) > W2_LOOKAHEAD:
                    emit_w2(*pend.pop(0))
        while pend:
            emit_w2(*pend.pop(0))
        w2_sb = []
        for qh in range(2):
            w2h = npool2.tile([Dh, S // 2], F32, tag="w2sb", bufs=4, name=f"w2sb_{it}_{j}_{qh}")
            nc.vector.tensor_copy(w2h[:], w2_ps[0:Dh, 1024 * qh : 1024 * (qh + 1)])
            w2_sb.append(w2h[:])

        def emit_rs():
            rs = []
            for qh in range(2):
                rsh = npool1.tile([1, S // 2], F32, tag="rs", bufs=4, name=f"rs_{it}_{j}_{qh}")
                nc.scalar.copy(rsh[:], w2_ps[Dh : Dh + 1, 1024 * qh : 1024 * (qh + 1)])
                rs.append(rsh[:])
            return rs

        return w2_sb, emit_rs

    def emit_ctx_prep(j, qh, rsh):
            rsr = npool1.tile([1, S // 2], F32, tag="rsr", bufs=2, name=f"rsr_{it}_{j}_{qh}")
            nc.vector.reciprocal_approx_fast(out=rsr[:], in_=rsh)
            rs_b = npool1.tile([Dh, S // 2], F32, tag="rs_b", bufs=2, name=f"rsb_{it}_{j}_{qh}")
            nc.gpsimd.partition_broadcast(rs_b[:], rsr[:])
            return rs_b

    def emit_ctx_finish(j, qh, w2h, rs_b):
            w2n_bf = npool2.tile([Dh, S // 2], BF16, tag="w2n", bufs=2, name=f"w2n_{it}_{j}_{qh}")
            nc.vector.tensor_mul(w2n_bf[:], w2h, rs_b[:])
            ctxT_bf = npool2.tile([Dh, S // 2], BF16, tag="ctxT", bufs=2, name=f"ctxT_{it}_{j}_{qh}")
            for t in range(2):
                c_ps = psum_big.tile([Dh, 512], F32, tag="big", name=f"cps_{it}_{j}_{qh}_{t}")
                nc.tensor.matmul(
                    c_ps[:], wvT_bf[:], w2n_bf[:, 512 * t : 512 * (t + 1)],
                    start=True, stop=True,
                )
                nc.vector.tensor_copy(ctxT_bf[:, 512 * t : 512 * (t + 1)], c_ps[:])
            if j < 2:
                nc.sync.dma_start(
                    out=in_cc[Dh * j : Dh * (j + 1), 1024 * qh : 1024 * (qh + 1)],
                    in_=ctxT_bf[:],
                )
            elif qh == 0:
                nc.sync.dma_start(
                    out=in_cc2h[qh][Dh * (j - 2) : Dh * (j - 1), :],
                    in_=ctxT_bf[:],
                )
            else:
                # half 1 goes to the contiguous quarter tiles feeding the
                # final quarter-granular AllGathers
                for t in range(2):
                    nc.sync.dma_start(
                        out=in_cc2q[t][Dh * (j - 2) : Dh * (j - 1), :],
                        in_=ctxT_bf[:, 512 * t : 512 * (t + 1)],
                    )

    def emit_ctx_half(j, qh, w2h, rsh):
        emit_ctx_finish(j, qh, w2h, emit_ctx_prep(j, qh, rsh))

    def emit_ctx(j, w2_sb, rs, after_half=None):
        """normalize + ctx matmuls + staging DMA (deferred one head),
        pipelined in q-halves to keep the serial chain short."""
        for qh in range(2):
            emit_ctx_half(j, qh, w2_sb[qh], rs[qh])
            if after_half is not None:
                after_half(qh)

    # software-pipelined head loop: head j's normalize/ctx is emitted after
    # head j+1's U projection so the in-order PE queue never head-of-line
    # blocks on the (DVE/GPSIMD) normalize chain.  The out projection is
    # split in two rounds around a split AllGather so most of it overlaps
    # the attention phase.
    agch = pool("agch", 1)
    # consolidated spread targets: one tile per AG event -> one spread DMA
    # instead of four (each DMA pays ~1.7us of issue+init latency)
    cch_ev = agch.tile([128, 4, S], BF16, tag="agev", name="agev")
    cch_od = [
        agch.tile([128, 4, S // 2], BF16, tag=f"agod{h}", name=f"agod{h}")
        for h in range(2)
    ]
    opool = pool("opool", 2)
    o_acc = [opool.tile([128, S], F32, tag=f"oacc{h}", bufs=1, name=f"oacc{h}") for h in range(2)]

    def emit_ag(which):
        """AllGather heads (0,1) [which=0] or (2,3) [which=1] of this batch."""
        if collective:
            nc.gpsimd.collective_compute(
                "AllGather",
                mybir.AluOpType.bypass,
                replica_groups=[[0, 1, 2, 3], [4, 5, 6, 7]],
                ins=[in_cc[:, :].opt()],
                outs=[ag_outs[which].opt()],
            )
        else:
            # sim stand-in: a light dep edge; real AG runs on TOPSP silicon
            nc.sync.dma_start(out=ag_outs[which][0:128, :], in_=in_cc[:, :])
        # chunk c8 = heads {2c8, 2c8+1}; AG round `which` supplies parity-
        # matching chunks: ag_outs[w] slab r = heads {4r+2w, 4r+2w+1} = chunk 2r+w
        assert which == 0
        nc.sync.dma_start(
            out=cch_ev[:], in_=ag_outs[which].rearrange("(r p) q -> p r q", p=128)
        )

    def emit_ag2(h):
        """AllGather heads (2,3), q-column half h only, so the tail pipelines."""
        if collective:
            nc.gpsimd.collective_compute(
                "AllGather",
                mybir.AluOpType.bypass,
                replica_groups=[[0, 1, 2, 3], [4, 5, 6, 7]],
                ins=[in_cc2h[h][:, :].opt()],
                outs=[ag2h[h].opt()],
            )
        else:
            nc.sync.dma_start(out=ag2h[h][0:128, :], in_=in_cc2h[h][:, :])
        nc.sync.dma_start(
            out=cch_od[h][:], in_=ag2h[h].rearrange("(r p) q -> p r q", p=128)
        )

    # contiguous quarter staging for the very last AG (collective inputs
    # must be contiguous, so column views of in_cc2h can't be used)
    in_cc2q = [
        dram.tile([2 * Dh, 512], BF16, name=f"incc2q_{it}_{t}", tag=f"incc2q{t}")
        for t in range(2)
    ]
    ag2q = [
        dram.tile([512, 512], BF16, addr_space="Local",
                  name=f"ag2q_{it}_{t}", tag=f"ag2q{t}")
        for t in range(2)
    ]

    def emit_ag2q(t):
        """AllGather heads (2,3), q-column quarter t of half 1."""
        if collective:
            nc.gpsimd.collective_compute(
                "AllGather",
                mybir.AluOpType.bypass,
                replica_groups=[[0, 1, 2, 3], [4, 5, 6, 7]],
                ins=[in_cc2q[t][:, :].opt()],
                outs=[ag2q[t].opt()],
            )
        else:
            nc.scalar.dma_start(out=ag2q[t][0:128, :], in_=in_cc2q[t][:, :])
        # spread issued from the (idle-at-tail) Pool queue: cheap issue slot
        # and no head-of-line blocking behind SP's earlier DMAs
        nc.scalar.dma_start(
            out=cch_od[1][:, :, 512 * t : 512 * (t + 1)],
            in_=ag2q[t].rearrange("(r p) q -> p r q", p=128),
        )

    def emit_oproj_group(round_, sh, h, acc_eng=None):
                o_ps = psum_big.tile([128, 1024], F32, tag="big", name=f"ops_{it}_{round_}_{h}_{sh}")
                for i, r in enumerate(range(4)):
                    c8 = 2 * r + round_
                    for u in range(2):
                        rhs = (
                            cch_ev[:, r, 1024 * sh + 512 * u : 1024 * sh + 512 * (u + 1)]
                            if round_ == 0
                            else cch_od[sh][:, r, 512 * u : 512 * (u + 1)]
                        )
                        nc.tensor.matmul(
                            o_ps[:, 512 * u : 512 * (u + 1)],
                            woT[c8][:, 128 * h : 128 * (h + 1)],
                            rhs,
                            start=(i == 0), stop=(i == 3),
                        )
                if round_ == 0:
                    (acc_eng or nc.vector.tensor_copy)(
                        o_acc[h][:, 1024 * sh : 1024 * (sh + 1)], o_ps[:]
                    )
                else:
                    o_sb = opool.tile([128, 1024], F32, tag="osb", name=f"osb_{it}_{h}_{sh}")
                    eng = nc.vector
                    eng.scalar_tensor_tensor(
                        o_sb[:], o_ps[:], bo_sb[:, h : h + 1],
                        o_acc[h][:, 1024 * sh : 1024 * (sh + 1)],
                        mybir.AluOpType.add, mybir.AluOpType.add,
                    )
                    nc.sync.dma_start(
                        out=outT[128 * h : 128 * (h + 1), 1024 * sh : 1024 * (sh + 1)],
                        in_=o_sb[:],
                    )

    def emit_oproj(round_):
        """Accumulate 4 chunks (parity `round_`) into o_acc (round 0) or
        finish with bias into outT (round 1)."""
        for sh in range(2):
            for h in range(2):
                emit_oproj_group(round_, sh, h)

    def emit_last_head(u_bf, prev_ctx, prev_rs):
        """Head 3 with q-half-outer loops: half 0's normalize/ctx/AG overlap
        half 1's attention, shrinking the serial tail."""
        j = HPC - 1
        w2_ps = psum_acc.tile([Dh + 1, S], F32, tag="acc", name=f"w2ps_{it}_last")
        halves = {}

        def attn_span(s0, s1, mids=None, post_first_exp=None, tail_emits=None):
            """Attention over 512-col q slices [s0, s1) of this head."""
            pend = []
            ns = s1 - s0

            def emit_w2(mm, pb):
                for u in range(ns):
                    nc.tensor.matmul(
                        w2_ps[:, 512 * (s0 + u) : 512 * (s0 + u + 1)],
                        vin_ones[:, mm, j, :],
                        pb[:, 512 * u : 512 * (u + 1)],
                        start=(mm == 0), stop=(mm == NK - 1),
                    )

            for m in range(NK):
                if mids is not None and m in mids:
                    mids[m]()
                kslice = kTh[j][:, 128 * m : 128 * (m + 1)]
                sc_ps = psum_big.tile([128, 512 * ns], F32, tag="big", name=f"scpsL_{s0}_{m}")
                for u in range(ns):
                    nc.tensor.matmul(
                        sc_ps[:, 512 * u : 512 * (u + 1)],
                        kslice,
                        u_bf[:, 512 * (s0 + u) : 512 * (s0 + u + 1)],
                        start=True, stop=True,
                    )
                p_bf = ppool.tile([128, 512 * ns], BF16, tag="p", name=f"pL_{s0}_{m}")
                nc.scalar.activation(
                    p_bf[:], sc_ps[:], mybir.ActivationFunctionType.Exp, scale=0.125
                )
                if m == 0 and post_first_exp is not None:
                    post_first_exp()
                pend.append((m, p_bf))
                if len(pend) > W2_LOOKAHEAD:
                    emit_w2(*pend.pop(0))
            while pend:
                emit_w2(*pend.pop(0))
                # interleaved tail work keeps PE hot (full p-state) through
                # the exp-bound last chunks
                if tail_emits:
                    tail_emits.pop(0)()

        def evac_half(qh):
            w2h = npool2.tile([Dh, S // 2], F32, tag="w2sb", bufs=4, name=f"w2sbL_{qh}")
            nc.vector.tensor_copy(w2h[:], w2_ps[0:Dh, 1024 * qh : 1024 * (qh + 1)])
            rsh = npool1.tile([1, S // 2], F32, tag="rs", bufs=4, name=f"rsL_{qh}")
            nc.scalar.copy(rsh[:], w2_ps[Dh : Dh + 1, 1024 * qh : 1024 * (qh + 1)])
            halves[qh] = (w2h[:], rsh[:])

        def tail_quarter_norm(t):
            """evac + normalize of q-column quarter t of half 1."""
            w2q = npool2.tile([Dh, 512], F32, tag="w2sb", bufs=4, name=f"w2qL_{t}")
            nc.vector.tensor_copy(w2q[:], w2_ps[0:Dh, 1024 + 512 * t : 1024 + 512 * (t + 1)])
            rsq = npool1.tile([1, 512], F32, tag="rs", bufs=4, name=f"rsqL_{t}")
            nc.scalar.copy(rsq[:], w2_ps[Dh : Dh + 1, 1024 + 512 * t : 1024 + 512 * (t + 1)])
            rsrq = npool1.tile([1, 512], F32, tag="rsr", bufs=2, name=f"rsrqL_{t}")
            nc.vector.reciprocal_approx_fast(out=rsrq[:], in_=rsq[:])
            rsbq = npool1.tile([Dh, 512], F32, tag="rs_b", bufs=2, name=f"rsbqL_{t}")
            nc.gpsimd.partition_broadcast(rsbq[:], rsrq[:])
            w2nq = npool2.tile([Dh, 512], BF16, tag="w2n", bufs=2, name=f"w2nqL_{t}")
            nc.vector.tensor_mul(w2nq[:], w2q[:], rsbq[:])
            return w2nq

        def tail_quarter_ctx(t, w2nq):
            c_ps = psum_big.tile([Dh, 512], F32, tag="big", name=f"cpsqL_{t}")
            nc.tensor.matmul(c_ps[:], wvT_bf[:], w2nq[:], start=True, stop=True)
            ctxq = npool2.tile([Dh, 512], BF16, tag="ctxT", bufs=2, name=f"ctxqL_{t}")
            nc.vector.tensor_copy(ctxq[:], c_ps[:])
            nc.scalar.dma_start(out=in_cc2q[t][Dh : 2 * Dh, :], in_=ctxq[:])
            emit_ag2q(t)

        def tail_quarter_oproj(t):
            for h in range(2):
                o_ps = psum_big.tile([128, 512], F32, tag="big", name=f"opsq_{t}_{h}")
                for r in range(4):
                    nc.tensor.matmul(
                        o_ps[:],
                        woT[2 * r + 1][:, 128 * h : 128 * (h + 1)],
                        cch_od[1][:, r, 512 * t : 512 * (t + 1)],
                        start=(r == 0), stop=(r == 3),
                    )
                o_sb = opool.tile([128, 512], F32, tag="osb", name=f"osbq_{t}_{h}")
                nc.vector.scalar_tensor_tensor(
                    o_sb[:], o_ps[:], bo_sb[:, h : h + 1],
                    o_acc[h][:, 1024 + 512 * t : 1024 + 512 * (t + 1)],
                    mybir.AluOpType.add, mybir.AluOpType.add,
                )
                nc.scalar.dma_start(
                    out=outT[128 * h : 128 * (h + 1), 1024 + 512 * t : 1024 + 512 * (t + 1)],
                    in_=o_sb[:],
                )

        attn_span(0, 2, mids={4: prev_ctx}, post_first_exp=prev_rs)
        evac_half(0)
        # half 1: spread oproj round 0 + half-0 ctx/AG across early chunks so
        # neither PE nor ACT sees a long detour at one point
        attn_span(2, 4, mids={
            1: lambda: [emit_oproj_group(0, 0, h) for h in range(2)],
            3: lambda: (emit_ctx_half(j, 0, *halves[0]), emit_ag2(0)),
            6: lambda: [emit_oproj_group(0, 1, h) for h in range(2)],
        })
        # quartered tail: launch each quarter's AG as early as possible and
        # interleave the long-ready oproj(1,0,*) groups as PE fill (also
        # keeping its p-state at full clock) while the DMA->AG chains drain
        wn0 = tail_quarter_norm(0)
        emit_oproj_group(1, 0, 0)
        tail_quarter_ctx(0, wn0)
        wn1 = tail_quarter_norm(1)
        emit_oproj_group(1, 0, 1)
        tail_quarter_ctx(1, wn1)
        tail_quarter_oproj(0)
        tail_quarter_oproj(1)

    u_next = [emit_u(0)]
    # v staging emitted after u(0) so its Pool-queue cast doesn't head-of-line
    # block u's psum evacuations (which gate the first scores)
    emit_stage_v(0)
    pending = None      # (j, w2_sb) of the previous head
    rs_store = {}       # j -> rs APs (filled by the deferred emitters)
    rs_emitters = {}    # j -> closure that emits the rs copies
    for j in range(HPC - 1):
        def mid():
            if j == 1:
                emit_stage_v(1)
                emit_wot(0)
            if j == 2:
                emit_wot(1)
            u_next.append(emit_u(j + 1))
        mids = {NK // 2: mid}
        if j == 0:
            mids[2] = emit_stage_k0b
        if j == 1:
            # g1 staging spread over head 1 (first needed by u(2) at m=8 /
            # head-2 scores) instead of one big detour inside head 0
            mids[2] = lambda: emit_stage_qk(qin, qT, "q", 1)
            mids[5] = lambda: emit_stage_qk(kin, kT, "k", 1)

        def post_fe(jp=j - 1):
            if jp in rs_emitters:
                rs_store[jp] = rs_emitters.pop(jp)()

        u_cur = u_next[-1]
        w2_sb, emit_rs = emit_scores_w2(
            j, u_cur, mid_emit=mids, post_first_exp=post_fe if j > 0 else None
        )
        rs_emitters[j] = emit_rs
        if pending is not None:
            jprev, w2_prev = pending
            emit_ctx(jprev, w2_prev, rs_store[jprev])
            if jprev == 1:
                emit_ag(0)
        pending = (j, w2_sb)
    jprev, w2_prev = pending
    emit_last_head(
        u_next[-1],
        prev_ctx=lambda: emit_ctx(jprev, w2_prev, rs_store[jprev]),
        prev_rs=lambda: rs_store.update({jprev: rs_emitters.pop(jprev)()}),
    )


def _build(repeats=1, collective=True):
    key = (repeats, collective)
    if key in _CACHE:
        return _CACHE[key]
    ndev = N_CORES if collective else 1
    nc = bacc.Bacc("TRN2", target_bir_lowering=False, debug=False, num_devices=ndev)
    io = _declare_io(nc)
    with tile.TileContext(nc) as tc:
        for it in range(repeats):
            with contextlib.ExitStack() as es:
                _body(nc, tc, es, io, it, collective=collective)
    nc.compile()
    _CACHE[key] = nc
    return nc


def kernel(k_in, q_in, v_in, Wq, Wk, Wv, Wo, bo, _repeats=1, _results_hook=None):
    k_in = np.asarray(k_in, dtype=np.float32)
    q_in = np.asarray(q_in, dtype=np.float32)
    v_in = np.asarray(v_in, dtype=np.float32)
    Wq = np.ascontiguousarray(np.asarray(Wq, dtype=np.float32))
    Wk = np.ascontiguousarray(np.asarray(Wk, dtype=np.float32))
    Wv = np.ascontiguousarray(np.asarray(Wv, dtype=np.float32))
    Wo = np.asarray(Wo, dtype=np.float32)
    bo = np.asarray(bo, dtype=np.float32)

    nc = _build(_repeats)

    in_maps = []
    for c in range(N_CORES):
        b, q4 = c // 4, c % 4
        sl = slice(256 * q4, 256 * (q4 + 1))
        in_maps.append(
            {
                "qin": np.ascontiguousarray(q_in[b, :, sl]),
                "kin": np.ascontiguousarray(k_in[b, :, sl]),
                "vin": np.ascontiguousarray(v_in[b, :, sl]),
                "wq": Wq,
                "wk": Wk,
                "wv": Wv,
                "wo_s": np.ascontiguousarray(Wo[sl, :]),
                "bo_s": np.ascontiguousarray(bo[sl].reshape(2, 128)),
            }
        )

    res = run_bass_kernel_spmd(nc, in_maps, core_ids=list(range(N_CORES)))
    if _results_hook is not None:
        _results_hook(res)

    out = np.empty((B, S, E), dtype=np.float32)
    for c in range(N_CORES):
        b, q4 = c // 4, c % 4
        out[b, :, 256 * q4 : 256 * (q4 + 1)] = res.results[c]["outT"].T
    return out



# revision 73
# speedup vs baseline: 1.0784x; 1.0127x over previous
"""Multi-head attention kernel for Trainium2, SPMD over 8 NeuronCores.

Problem: B=2, S=2048, E=1024, H=16 heads, Dh=64.
  q = per-head q_in @ Wq.T (Wq shared across heads), same for k, v
  attn = softmax(q k^T / 8); ctx = attn @ v; out = concat(ctx) @ Wo.T + bo

Sharding: core c handles batch b=c//4 and heads 4*(c%4)..4*(c%4)+3
(head-parallel attention).  The out projection is sharded by e_out columns
(each core receives 256 rows of Wo, host-sliced), with an AllGather of the
per-head context over the 4 cores of each batch group in between.

All matmuls run in bf16 with fp32 PSUM accumulation; softmax statistics
(row sums / reciprocals) stay fp32.

Layout tricks (avoid transposing activations for the V path):
  scores^T = kin @ (A @ qin^T)     with A = Wq^T Wk (projection fused)
  ctx^T    = Wv @ (vin^T @ P^T)    (vin used in natural layout)
  rowsum   = extra ones-column on vin (rides the PE contraction for free)
"""

import contextlib
import sys

sys.path.insert(0, "/opt/trn_rl_repo")

import numpy as np

import concourse.bass as bass
import concourse.masks as masks
import concourse.tile as tile
from concourse import bacc, mybir
from concourse.bass_utils import run_bass_kernel_spmd

B, S, E, H, Dh = 2, 2048, 1024, 16, 64
N_CORES = 8
HPC = 4          # heads per core
NK = S // 128    # 16 key chunks
EOUT = E // 4    # e_out columns per core

F32 = mybir.dt.float32
BF16 = mybir.dt.bfloat16

_CACHE = {}


def _declare_io(nc):
    io = {}
    io["qin"] = nc.dram_tensor("qin", [S, HPC * Dh], F32, kind="ExternalInput").ap()
    io["kin"] = nc.dram_tensor("kin", [S, HPC * Dh], F32, kind="ExternalInput").ap()
    io["vin"] = nc.dram_tensor("vin", [S, HPC * Dh], F32, kind="ExternalInput").ap()
    io["wq"] = nc.dram_tensor("wq", [Dh, Dh], F32, kind="ExternalInput").ap()
    io["wk"] = nc.dram_tensor("wk", [Dh, Dh], F32, kind="ExternalInput").ap()
    io["wv"] = nc.dram_tensor("wv", [Dh, Dh], F32, kind="ExternalInput").ap()
    io["wo_s"] = nc.dram_tensor("wo_s", [EOUT, E], F32, kind="ExternalInput").ap()
    io["bo_s"] = nc.dram_tensor("bo_s", [2, 128], F32, kind="ExternalInput").ap()
    io["outT"] = nc.dram_tensor("outT", [EOUT, S], F32, kind="ExternalOutput").ap()
    return io


def _body(nc, tc, es, io, it, collective=True):
    """One full MHA iteration. `it` only namespaces pool names."""

    def pool(name, bufs, space="SBUF"):
        return es.enter_context(
            tc.tile_pool(name=f"{name}_{it}", bufs=bufs, space=space)
        )

    qin, kin, vin = io["qin"], io["kin"], io["vin"]
    wq, wk, wv, wo_s, bo_s, outT = (
        io["wq"], io["wk"], io["wv"], io["wo_s"], io["bo_s"], io["outT"],
    )

    stage = pool("stage", 2)          # fp32/bf16 staging for casts
    persist = pool("persist", 1)      # long-lived bf16 tensors
    psum_big = pool("psum_big", 2, space="PSUM")    # [128,1024] = 2 banks x2
    psum_acc = pool("psum_acc", 1, space="PSUM")    # [*, 2048]  = 4 banks x1
    upool = pool("upool", 2)
    ppool = pool("ppool", 6)
    npool1 = pool("npool1", 1)        # rsr / rs_b (rs gets 2 bufs below)
    npool2 = pool("npool2", 2)        # w2n / ctxT
    dram = pool("dram", 1, space="DRAM")

    # identity for PE transposes
    ident = persist.tile([128, 128], F32, tag="ident")
    masks.make_identity(nc, ident[:])

    # ---------------- tiny weight prep ----------------
    wq_sb = persist.tile([Dh, Dh], F32, tag="wq_sb")
    nc.sync.dma_start(out=wq_sb[:], in_=wq[:, :])
    wk_sb = persist.tile([Dh, Dh], F32, tag="wk_sb")
    nc.sync.dma_start(out=wk_sb[:], in_=wk[:, :])
    wq_bf = persist.tile([Dh, Dh], BF16, tag="wq_bf")
    nc.vector.tensor_copy(wq_bf[:], wq_sb[:])
    wk_bf = persist.tile([Dh, Dh], BF16, tag="wk_bf")
    nc.vector.tensor_copy(wk_bf[:], wk_sb[:])

    # A = Wq^T @ Wk   [64,64]
    a_ps = psum_big.tile([Dh, Dh], F32, tag="big")
    nc.tensor.matmul(a_ps[:], wq_bf[:], wk_bf[:], start=True, stop=True)
    a_bf = persist.tile([Dh, Dh], BF16, tag="a_bf")
    nc.vector.tensor_copy(a_bf[:], a_ps[:])

    # WvT = Wv^T via small strided DMA from DRAM (64x64, one-time)
    wvT_sb = persist.tile([Dh, Dh], F32, tag="wvT_sb")
    nc.sync.dma_start(out=wvT_sb[:], in_=wv.rearrange("a b -> b a"))
    wvT_bf = persist.tile([Dh, Dh], BF16, tag="wvT_bf")
    nc.vector.tensor_copy(wvT_bf[:], wvT_sb[:])

    # ---------------- activations: load + PE block-transpose ----------------
    # qT/kT packs: [128, 2048] bf16; pack g holds heads 2g (rows 0-63), 2g+1 (64-127)
    qT = [persist.tile([128, S], BF16, tag=f"qT{g}", name=f"qT{g}") for g in range(2)]
    kT = [persist.tile([128, S], BF16, tag=f"kT{g}", name=f"kT{g}") for g in range(2)]
    # per-head base-partition-0 views; odd heads are DMA-copied after transpose
    hsplit = {}
    for hh in ("q", "k"):
        for j in (1, 3):
            hsplit[(hh, j)] = persist.tile(
                [Dh, S], BF16, tag=f"hsplit_{hh}{j}", name=f"hsplit_{it}_{hh}{j}"
            )
    qTh = [qT[0][0:Dh, :], hsplit[("q", 1)][:], qT[1][0:Dh, :], hsplit[("q", 3)][:]]
    kTh = [kT[0][0:Dh, :], hsplit[("k", 1)][:], kT[1][0:Dh, :], hsplit[("k", 3)][:]]

    # vin_ones: [128, NK, HPC, 65] bf16 (col 64 = 1.0 rides the contraction)
    vin_ones = persist.tile([128, NK, HPC, Dh + 1], BF16, tag="vin_ones")
    nc.vector.memset(vin_ones[:, :, :, Dh : Dh + 1], 1.0)

    qk_stage_tiles = {}

    def emit_stage_qk(src_ap, packs, hh, g, c0=0, c1=NK, hsplit_dma=True):
        """Load chunks [c0,c1) of one [2048,128] half, PE-transpose into the
        pack, and (once the pack is complete) split off the odd head.
        Chunk-ranged so k staging can be split around the first scores."""
        key = (hh, g)
        if key not in qk_stage_tiles:
            # k g=0 is filled by two chunk-ranged calls; pin it to its own
            # buffer so the pool can't recycle it between the calls
            tag, bufs = ("kst0", 1) if key == ("k", 0) else ("astage", 2)
            qk_stage_tiles[key] = stage.tile(
                [128, NK, 128], F32, tag=tag, bufs=bufs, name=f"st_{it}_{hh}{g}"
            )
        st = qk_stage_tiles[key]
        src_r = src_ap[:, 128 * g : 128 * (g + 1)].rearrange("(c p) d -> p c d", p=128)
        for cb in range(c0, c1, 4):  # 4-chunk loads so transposes start early
            ce = min(cb + 4, c1)
            nc.sync.dma_start(out=st[:, cb:ce, :], in_=src_r[:, cb:ce, :])
        for cq in range(c0 // 4, c1 // 4):  # 4 transposes batched per psum bank
            t_ps = psum_big.tile([128, 512], F32, tag="big", name=f"tp_{it}_{hh}{g}_{cq}")
            for ci in range(4):
                c = 4 * cq + ci
                nc.tensor.transpose(
                    t_ps[:, 128 * ci : 128 * (ci + 1)], st[:, c, :], ident[:]
                )
            nc.vector.tensor_copy(packs[g][:, 512 * cq : 512 * (cq + 1)], t_ps[:])
        if hsplit_dma:
            nc.sync.dma_start(
                out=hsplit[(hh, 2 * g + 1)][:], in_=packs[g][Dh : 2 * Dh, :]
            )

    def emit_stage_v(g):
        st = stage.tile([128, NK, 128], F32, tag="astage", name=f"stv_{it}_{g}")
        nc.sync.dma_start(
            out=st[:],
            in_=vin[:, 128 * g : 128 * (g + 1)].rearrange("(c p) d -> p c d", p=128),
        )
        # DVE for both casts: ACT must stay clear for exp (in-order queue:
        # anything enqueued before exp delays it)
        eng = nc.vector.tensor_copy
        eng(
            vin_ones[:, :, 2 * g : 2 * g + 2, 0:Dh],
            st[:].rearrange("p c (jj d) -> p c jj d", jj=2),
        )

    # k chunks 0-7 first (scores m=0.. need them before anything else in the
    # in-order PE queue), then the full q pack (u needs all 16 chunks), then
    # v.  k chunks 8-15 are staged inside head 0 (mid hook at m=2).
    emit_stage_qk(kin, kT, "k", 0, 0, NK // 2, hsplit_dma=False)
    emit_stage_qk(qin, qT, "q", 0)

    def emit_stage_k0b():
        emit_stage_qk(kin, kT, "k", 0, NK // 2, NK)

    def emit_stage_g1():
        emit_stage_qk(qin, qT, "q", 1)
        emit_stage_qk(kin, kT, "k", 1)

    # single [128, c8, EOUT] tile: the transpose evacuation then needs one
    # strided DVE copy per 4-chunk batch instead of four narrow ones
    woT = persist.tile([128, 8, EOUT], BF16, tag="woT", name="woT")
    bo_sb = persist.tile([128, 2], F32, tag="bo_sb2")

    def emit_wot(r):
        if r == 0:
            for h in range(2):
                nc.sync.dma_start(
                    out=bo_sb[:, h : h + 1],
                    in_=bo_s[h, :].rearrange("(p one) -> p one", one=1),
                )
        w_st = stage.tile([128, E], F32, tag="wostage", bufs=1, name=f"wst_{it}_{r}")
        nc.sync.dma_start(out=w_st[:], in_=wo_s[128 * r : 128 * (r + 1), :])
        for q8 in range(2):  # 4 transposes batched through one psum bank
            t_ps = psum_big.tile([128, 512], F32, tag="big", name=f"wtp_{it}_{r}_{q8}")
            for ci in range(4):
                c8 = 4 * q8 + ci
                nc.tensor.transpose(
                    t_ps[:, 128 * ci : 128 * (ci + 1)], w_st[:, 128 * c8 : 128 * (c8 + 1)], ident[:]
                )
            nc.vector.tensor_copy(
                woT[:, 4 * q8 : 4 * (q8 + 1), 128 * r : 128 * (r + 1)],
                t_ps[:].rearrange("p (c x) -> p c x", c=4),
            )

    # ---------------- attention per head ----------------
    in_cc = dram.tile([2 * Dh, S], BF16)  # heads 0,1 (AG round 0)
    in_cc2h = [
        dram.tile([2 * Dh, S // 2], BF16, name=f"incc2_{it}_{h}", tag=f"incc2{h}")
        for h in range(2)
    ]  # heads 2,3 staged per q-half, contiguous for the split AG
    ag_outs = [
        dram.tile(
            [512, S], BF16,
            addr_space="Local",
            name=f"agout_{it}_{w}", tag=f"agout{w}",
        )
        for w in range(2)
    ]
    ag2h = [
        dram.tile([512, S // 2], BF16, addr_space="Local",
                  name=f"ag2h_{it}_{h}", tag=f"ag2h{h}")
        for h in range(2)
    ]

    def emit_u(j):
        u_bf = upool.tile([Dh, S], BF16, tag="u", name=f"u_{it}_{j}")
        for t in range(4):
            u_ps = psum_big.tile([Dh, 512], F32, tag="big", name=f"ups_{it}_{j}_{t}")
            nc.tensor.matmul(
                u_ps[:], a_bf[:], qTh[j][:, 512 * t : 512 * (t + 1)],
                start=True, stop=True,
            )
            nc.vector.tensor_copy(u_bf[:, 512 * t : 512 * (t + 1)], u_ps[:])
        return u_bf

    W2_LOOKAHEAD = 3  # score/exp chunk-halves emitted ahead of their W2

    def emit_scores_w2(j, u_bf, mid_emit=None, post_first_exp=None):
        """scores -> exp -> W2 accumulation, then eager psum evacuation.
        W2 emission trails the scores/exp stream by W2_LOOKAHEAD halves so
        the in-order PE queue never head-of-line blocks on psum_acc reuse
        (its evacuation overlaps the next head's first scores).  `mid_emit`
        is a dict {m: callback} fired before chunk m.  Returns (w2_sb, rs)."""
        w2_ps = psum_acc.tile([Dh + 1, S], F32, tag="acc", name=f"w2ps_{it}_{j}")
        pend = []

        def emit_w2(mm, qq, pb):
            for u in range(2):
                nc.tensor.matmul(
                    w2_ps[:, 1024 * qq + 512 * u : 1024 * qq + 512 * (u + 1)],
                    vin_ones[:, mm, j, :],
                    pb[:, 512 * u : 512 * (u + 1)],
                    start=(mm == 0), stop=(mm == NK - 1),
                )

        for m in range(NK):
            if mid_emit is not None and m in mid_emit:
                mid_emit[m]()
            kslice = kTh[j][:, 128 * m : 128 * (m + 1)]
            for qh in range(2):
                sc_ps = psum_big.tile([128, 1024], F32, tag="big", name=f"scps_{it}_{j}_{m}_{qh}")
                for u in range(2):
                    nc.tensor.matmul(
                        sc_ps[:, 512 * u : 512 * (u + 1)],
                        kslice,
                        u_bf[:, 1024 * qh + 512 * u : 1024 * qh + 512 * (u + 1)],
                        start=True, stop=True,
                    )
                p_bf = ppool.tile([128, 1024], BF16, tag="p", name=f"p_{it}_{j}_{m}_{qh}")
                nc.scalar.activation(
                    p_bf[:], sc_ps[:], mybir.ActivationFunctionType.Exp, scale=0.125
                )
                if m == 0 and qh == 0 and post_first_exp is not None:
                    # previous head's rs copies slot in here: ACT would
                    # otherwise idle waiting for this head's next scores
                    post_first_exp()
                pend.append((m, qh, p_bf))
                if len(pend) > W2_LOOKAHEAD:
                    emit_w2(*pend.pop(0))
        while pend:
            emit_w2(*pend.pop(0))
        w2_sb = []
        for qh in range(2):
            w2h = npool2.tile([Dh, S // 2], F32, tag="w2sb", bufs=4, name=f"w2sb_{it}_{j}_{qh}")
            nc.vector.tensor_copy(w2h[:], w2_ps[0:Dh, 1024 * qh : 1024 * (qh + 1)])
            w2_sb.append(w2h[:])

        def emit_rs():
            rs = []
            for qh in range(2):
                rsh = npool1.tile([1, S // 2], F32, tag="rs", bufs=4, name=f"rs_{it}_{j}_{qh}")
                nc.scalar.copy(rsh[:], w2_ps[Dh : Dh + 1, 1024 * qh : 1024 * (qh + 1)])
                rs.append(rsh[:])
            return rs

        return w2_sb, emit_rs

    def emit_ctx_prep(j, qh, rsh):
            rsr = npool1.tile([1, S // 2], F32, tag="rsr", bufs=2, name=f"rsr_{it}_{j}_{qh}")
            nc.vector.reciprocal_approx_fast(out=rsr[:], in_=rsh)
            rs_b = npool1.tile([Dh, S // 2], F32, tag="rs_b", bufs=2, name=f"rsb_{it}_{j}_{qh}")
            nc.gpsimd.partition_broadcast(rs_b[:], rsr[:])
            return rs_b

    def emit_ctx_finish(j, qh, w2h, rs_b):
            w2n_bf = npool2.tile([Dh, S // 2], BF16, tag="w2n", bufs=2, name=f"w2n_{it}_{j}_{qh}")
            nc.vector.tensor_mul(w2n_bf[:], w2h, rs_b[:])
            ctxT_bf = npool2.tile([Dh, S // 2], BF16, tag="ctxT", bufs=2, name=f"ctxT_{it}_{j}_{qh}")
            for t in range(2):
                c_ps = psum_big.tile([Dh, 512], F32, tag="big", name=f"cps_{it}_{j}_{qh}_{t}")
                nc.tensor.matmul(
                    c_ps[:], wvT_bf[:], w2n_bf[:, 512 * t : 512 * (t + 1)],
                    start=True, stop=True,
                )
                nc.vector.tensor_copy(ctxT_bf[:, 512 * t : 512 * (t + 1)], c_ps[:])
            if j < 2:
                nc.sync.dma_start(
                    out=in_cc[Dh * j : Dh * (j + 1), 1024 * qh : 1024 * (qh + 1)],
                    in_=ctxT_bf[:],
                )
            elif qh == 0:
                nc.sync.dma_start(
                    out=in_cc2h[qh][Dh * (j - 2) : Dh * (j - 1), :],
                    in_=ctxT_bf[:],
                )
            else:
                # half 1 goes to the contiguous quarter tiles feeding the
                # final quarter-granular AllGathers
                for t in range(2):
                    nc.sync.dma_start(
                        out=in_cc2q[t][Dh * (j - 2) : Dh * (j - 1), :],
                        in_=ctxT_bf[:, 512 * t : 512 * (t + 1)],
                    )

    def emit_ctx_half(j, qh, w2h, rsh):
        emit_ctx_finish(j, qh, w2h, emit_ctx_prep(j, qh, rsh))

    def emit_ctx(j, w2_sb, rs, after_half=None):
        """normalize + ctx matmuls + staging DMA (deferred one head),
        pipelined in q-halves to keep the serial chain short."""
        for qh in range(2):
            emit_ctx_half(j, qh, w2_sb[qh], rs[qh])
            if after_half is not None:
                after_half(qh)

    # software-pipelined head loop: head j's normalize/ctx is emitted after
    # head j+1's U projection so the in-order PE queue never head-of-line
    # blocks on the (DVE/GPSIMD) normalize chain.  The out projection is
    # split in two rounds around a split AllGather so most of it overlaps
    # the attention phase.
    agch = pool("agch", 1)
    # consolidated spread targets: one tile per AG event -> one spread DMA
    # instead of four (each DMA pays ~1.7us of issue+init latency)
    cch_ev = agch.tile([128, 4, S], BF16, tag="agev", name="agev")
    cch_od = [
        agch.tile([128, 4, S // 2], BF16, tag=f"agod{h}", name=f"agod{h}")
        for h in range(2)
    ]
    opool = pool("opool", 2)
    o_acc = [opool.tile([128, S], F32, tag=f"oacc{h}", bufs=1, name=f"oacc{h}") for h in range(2)]

    def emit_ag(which):
        """AllGather heads (0,1) [which=0] or (2,3) [which=1] of this batch."""
        if collective:
            nc.gpsimd.collective_compute(
                "AllGather",
                mybir.AluOpType.bypass,
                replica_groups=[[0, 1, 2, 3], [4, 5, 6, 7]],
                ins=[in_cc[:, :].opt()],
                outs=[ag_outs[which].opt()],
            )
        else:
            # sim stand-in: a light dep edge; real AG runs on TOPSP silicon
            nc.sync.dma_start(out=ag_outs[which][0:128, :], in_=in_cc[:, :])
        # chunk c8 = heads {2c8, 2c8+1}; AG round `which` supplies parity-
        # matching chunks: ag_outs[w] slab r = heads {4r+2w, 4r+2w+1} = chunk 2r+w
        assert which == 0
        nc.sync.dma_start(
            out=cch_ev[:], in_=ag_outs[which].rearrange("(r p) q -> p r q", p=128)
        )

    def emit_ag2(h):
        """AllGather heads (2,3), q-column half h only, so the tail pipelines."""
        if collective:
            nc.gpsimd.collective_compute(
                "AllGather",
                mybir.AluOpType.bypass,
                replica_groups=[[0, 1, 2, 3], [4, 5, 6, 7]],
                ins=[in_cc2h[h][:, :].opt()],
                outs=[ag2h[h].opt()],
            )
        else:
            nc.sync.dma_start(out=ag2h[h][0:128, :], in_=in_cc2h[h][:, :])
        nc.sync.dma_start(
            out=cch_od[h][:], in_=ag2h[h].rearrange("(r p) q -> p r q", p=128)
        )

    # contiguous quarter staging for the very last AG (collective inputs
    # must be contiguous, so column views of in_cc2h can't be used)
    in_cc2q = [
        dram.tile([2 * Dh, 512], BF16, name=f"incc2q_{it}_{t}", tag=f"incc2q{t}")
        for t in range(2)
    ]
    ag2q = [
        dram.tile([512, 512], BF16, addr_space="Local",
                  name=f"ag2q_{it}_{t}", tag=f"ag2q{t}")
        for t in range(2)
    ]

    def emit_ag2q(t):
        """AllGather heads (2,3), q-column quarter t of half 1."""
        if collective:
            nc.gpsimd.collective_compute(
                "AllGather",
                mybir.AluOpType.bypass,
                replica_groups=[[0, 1, 2, 3], [4, 5, 6, 7]],
                ins=[in_cc2q[t][:, :].opt()],
                outs=[ag2q[t].opt()],
            )
        else:
            nc.scalar.dma_start(out=ag2q[t][0:128, :], in_=in_cc2q[t][:, :])
        # spread issued from the (idle-at-tail) Pool queue: cheap issue slot
        # and no head-of-line blocking behind SP's earlier DMAs
        nc.scalar.dma_start(
            out=cch_od[1][:, :, 512 * t : 512 * (t + 1)],
            in_=ag2q[t].rearrange("(r p) q -> p r q", p=128),
        )

    def emit_oproj_group(round_, sh, h, acc_eng=None):
                o_ps = psum_big.tile([128, 1024], F32, tag="big", name=f"ops_{it}_{round_}_{h}_{sh}")
                for i, r in enumerate(range(4)):
                    c8 = 2 * r + round_
                    for u in range(2):
                        rhs = (
                            cch_ev[:, r, 1024 * sh + 512 * u : 1024 * sh + 512 * (u + 1)]
                            if round_ == 0
                            else cch_od[sh][:, r, 512 * u : 512 * (u + 1)]
                        )
                        nc.tensor.matmul(
                            o_ps[:, 512 * u : 512 * (u + 1)],
                            woT[:, c8, 128 * h : 128 * (h + 1)],
                            rhs,
                            start=(i == 0), stop=(i == 3),
                        )
                if round_ == 0:
                    (acc_eng or nc.vector.tensor_copy)(
                        o_acc[h][:, 1024 * sh : 1024 * (sh + 1)], o_ps[:]
                    )
                else:
                    o_sb = opool.tile([128, 1024], F32, tag="osb", name=f"osb_{it}_{h}_{sh}")
                    eng = nc.vector
                    eng.scalar_tensor_tensor(
                        o_sb[:], o_ps[:], bo_sb[:, h : h + 1],
                        o_acc[h][:, 1024 * sh : 1024 * (sh + 1)],
                        mybir.AluOpType.add, mybir.AluOpType.add,
                    )
                    nc.sync.dma_start(
                        out=outT[128 * h : 128 * (h + 1), 1024 * sh : 1024 * (sh + 1)],
                        in_=o_sb[:],
                    )

    def emit_oproj(round_):
        """Accumulate 4 chunks (parity `round_`) into o_acc (round 0) or
        finish with bias into outT (round 1)."""
        for sh in range(2):
            for h in range(2):
                emit_oproj_group(round_, sh, h)

    def emit_last_head(u_bf, prev_ctx, prev_rs):
        """Head 3 with q-half-outer loops: half 0's normalize/ctx/AG overlap
        half 1's attention, shrinking the serial tail."""
        j = HPC - 1
        w2_ps = psum_acc.tile([Dh + 1, S], F32, tag="acc", name=f"w2ps_{it}_last")
        halves = {}

        def attn_span(s0, s1, mids=None, post_first_exp=None, tail_emits=None):
            """Attention over 512-col q slices [s0, s1) of this head."""
            pend = []
            ns = s1 - s0

            def emit_w2(mm, pb):
                for u in range(ns):
                    nc.tensor.matmul(
                        w2_ps[:, 512 * (s0 + u) : 512 * (s0 + u + 1)],
                        vin_ones[:, mm, j, :],
                        pb[:, 512 * u : 512 * (u + 1)],
                        start=(mm == 0), stop=(mm == NK - 1),
                    )

            for m in range(NK):
                if mids is not None and m in mids:
                    mids[m]()
                kslice = kTh[j][:, 128 * m : 128 * (m + 1)]
                sc_ps = psum_big.tile([128, 512 * ns], F32, tag="big", name=f"scpsL_{s0}_{m}")
                for u in range(ns):
                    nc.tensor.matmul(
                        sc_ps[:, 512 * u : 512 * (u + 1)],
                        kslice,
                        u_bf[:, 512 * (s0 + u) : 512 * (s0 + u + 1)],
                        start=True, stop=True,
                    )
                p_bf = ppool.tile([128, 512 * ns], BF16, tag="p", name=f"pL_{s0}_{m}")
                nc.scalar.activation(
                    p_bf[:], sc_ps[:], mybir.ActivationFunctionType.Exp, scale=0.125
                )
                if m == 0 and post_first_exp is not None:
                    post_first_exp()
                pend.append((m, p_bf))
                if len(pend) > W2_LOOKAHEAD:
                    emit_w2(*pend.pop(0))
            while pend:
                emit_w2(*pend.pop(0))
                # interleaved tail work keeps PE hot (full p-state) through
                # the exp-bound last chunks
                if tail_emits:
                    tail_emits.pop(0)()

        def evac_half(qh):
            w2h = npool2.tile([Dh, S // 2], F32, tag="w2sb", bufs=4, name=f"w2sbL_{qh}")
            nc.vector.tensor_copy(w2h[:], w2_ps[0:Dh, 1024 * qh : 1024 * (qh + 1)])
            rsh = npool1.tile([1, S // 2], F32, tag="rs", bufs=4, name=f"rsL_{qh}")
            nc.scalar.copy(rsh[:], w2_ps[Dh : Dh + 1, 1024 * qh : 1024 * (qh + 1)])
            halves[qh] = (w2h[:], rsh[:])

        def tail_quarter_norm(t):
            """evac + normalize of q-column quarter t of half 1."""
            w2q = npool2.tile([Dh, 512], F32, tag="w2sb", bufs=4, name=f"w2qL_{t}")
            nc.vector.tensor_copy(w2q[:], w2_ps[0:Dh, 1024 + 512 * t : 1024 + 512 * (t + 1)])
            rsq = npool1.tile([1, 512], F32, tag="rs", bufs=4, name=f"rsqL_{t}")
            nc.scalar.copy(rsq[:], w2_ps[Dh : Dh + 1, 1024 + 512 * t : 1024 + 512 * (t + 1)])
            rsrq = npool1.tile([1, 512], F32, tag="rsr", bufs=2, name=f"rsrqL_{t}")
            nc.vector.reciprocal_approx_fast(out=rsrq[:], in_=rsq[:])
            rsbq = npool1.tile([Dh, 512], F32, tag="rs_b", bufs=2, name=f"rsbqL_{t}")
            nc.gpsimd.partition_broadcast(rsbq[:], rsrq[:])
            w2nq = npool2.tile([Dh, 512], BF16, tag="w2n", bufs=2, name=f"w2nqL_{t}")
            nc.vector.tensor_mul(w2nq[:], w2q[:], rsbq[:])
            return w2nq

        def tail_quarter_ctx(t, w2nq):
            c_ps = psum_big.tile([Dh, 512], F32, tag="big", name=f"cpsqL_{t}")
            nc.tensor.matmul(c_ps[:], wvT_bf[:], w2nq[:], start=True, stop=True)
            ctxq = npool2.tile([Dh, 512], BF16, tag="ctxT", bufs=2, name=f"ctxqL_{t}")
            nc.vector.tensor_copy(ctxq[:], c_ps[:])
            nc.scalar.dma_start(out=in_cc2q[t][Dh : 2 * Dh, :], in_=ctxq[:])
            emit_ag2q(t)

        def tail_quarter_oproj(t):
            for h in range(2):
                o_ps = psum_big.tile([128, 512], F32, tag="big", name=f"opsq_{t}_{h}")
                for r in range(4):
                    nc.tensor.matmul(
                        o_ps[:],
                        woT[:, 2 * r + 1, 128 * h : 128 * (h + 1)],
                        cch_od[1][:, r, 512 * t : 512 * (t + 1)],
                        start=(r == 0), stop=(r == 3),
                    )
                o_sb = opool.tile([128, 512], F32, tag="osb", name=f"osbq_{t}_{h}")
                nc.vector.scalar_tensor_tensor(
                    o_sb[:], o_ps[:], bo_sb[:, h : h + 1],
                    o_acc[h][:, 1024 + 512 * t : 1024 + 512 * (t + 1)],
                    mybir.AluOpType.add, mybir.AluOpType.add,
                )
                nc.scalar.dma_start(
                    out=outT[128 * h : 128 * (h + 1), 1024 + 512 * t : 1024 + 512 * (t + 1)],
                    in_=o_sb[:],
                )

        attn_span(0, 2, mids={4: prev_ctx}, post_first_exp=prev_rs)
        evac_half(0)
        # half 1: spread oproj round 0 + half-0 ctx/AG across early chunks so
        # neither PE nor ACT sees a long detour at one point
        attn_span(2, 4, mids={
            1: lambda: [emit_oproj_group(0, 0, h) for h in range(2)],
            3: lambda: (emit_ctx_half(j, 0, *halves[0]), emit_ag2(0)),
            6: lambda: [emit_oproj_group(0, 1, h) for h in range(2)],
        })
        # quartered tail: launch each quarter's AG as early as possible and
        # interleave the long-ready oproj(1,0,*) groups as PE fill (also
        # keeping its p-state at full clock) while the DMA->AG chains drain
        wn0 = tail_quarter_norm(0)
        emit_oproj_group(1, 0, 0)
        tail_quarter_ctx(0, wn0)
        wn1 = tail_quarter_norm(1)
        emit_oproj_group(1, 0, 1)
        tail_quarter_ctx(1, wn1)
        tail_quarter_oproj(0)
        tail_quarter_oproj(1)

    u_next = [emit_u(0)]
    # v staging emitted after u(0) so its Pool-queue cast doesn't head-of-line
    # block u's psum evacuations (which gate the first scores)
    emit_stage_v(0)
    pending = None      # (j, w2_sb) of the previous head
    rs_store = {}       # j -> rs APs (filled by the deferred emitters)
    rs_emitters = {}    # j -> closure that emits the rs copies
    for j in range(HPC - 1):
        mids = {NK // 2: lambda: u_next.append(emit_u(j + 1))}
        if j == 0:
            mids[2] = emit_stage_k0b
            mids[12] = lambda: emit_wot(0)
        if j == 1:
            # g1 staging spread over head 1 (first needed by u(2) at m=8 /
            # head-2 scores) instead of one big detour inside head 0
            mids[2] = lambda: emit_stage_qk(qin, qT, "q", 1)
            mids[5] = lambda: emit_stage_qk(kin, kT, "k", 1)
            mids[11] = lambda: emit_stage_v(1)
        if j == 2:
            mids[5] = lambda: emit_wot(1)

        def post_fe(jp=j - 1):
            if jp in rs_emitters:
                rs_store[jp] = rs_emitters.pop(jp)()

        u_cur = u_next[-1]
        w2_sb, emit_rs = emit_scores_w2(
            j, u_cur, mid_emit=mids, post_first_exp=post_fe if j > 0 else None
        )
        rs_emitters[j] = emit_rs
        if pending is not None:
            jprev, w2_prev = pending
            emit_ctx(jprev, w2_prev, rs_store[jprev])
            if jprev == 1:
                emit_ag(0)
        pending = (j, w2_sb)
    jprev, w2_prev = pending
    emit_last_head(
        u_next[-1],
        prev_ctx=lambda: emit_ctx(jprev, w2_prev, rs_store[jprev]),
        prev_rs=lambda: rs_store.update({jprev: rs_emitters.pop(jprev)()}),
    )


def _build(repeats=1, collective=True):
    key = (repeats, collective)
    if key in _CACHE:
        return _CACHE[key]
    ndev = N_CORES if collective else 1
    nc = bacc.Bacc("TRN2", target_bir_lowering=False, debug=False, num_devices=ndev)
    io = _declare_io(nc)
    with tile.TileContext(nc) as tc:
        for it in range(repeats):
            with contextlib.ExitStack() as es:
                _body(nc, tc, es, io, it, collective=collective)
    nc.compile()
    _CACHE[key] = nc
    return nc


def kernel(k_in, q_in, v_in, Wq, Wk, Wv, Wo, bo, _repeats=1, _results_hook=None):
    k_in = np.asarray(k_in, dtype=np.float32)
    q_in = np.asarray(q_in, dtype=np.float32)
    v_in = np.asarray(v_in, dtype=np.float32)
    Wq = np.ascontiguousarray(np.asarray(Wq, dtype=np.float32))
    Wk = np.ascontiguousarray(np.asarray(Wk, dtype=np.float32))
    Wv = np.ascontiguousarray(np.asarray(Wv, dtype=np.float32))
    Wo = np.asarray(Wo, dtype=np.float32)
    bo = np.asarray(bo, dtype=np.float32)

    nc = _build(_repeats)

    in_maps = []
    for c in range(N_CORES):
        b, q4 = c // 4, c % 4
        sl = slice(256 * q4, 256 * (q4 + 1))
        in_maps.append(
            {
                "qin": np.ascontiguousarray(q_in[b, :, sl]),
                "kin": np.ascontiguousarray(k_in[b, :, sl]),
                "vin": np.ascontiguousarray(v_in[b, :, sl]),
                "wq": Wq,
                "wk": Wk,
                "wv": Wv,
                "wo_s": np.ascontiguousarray(Wo[sl, :]),
                "bo_s": np.ascontiguousarray(bo[sl].reshape(2, 128)),
            }
        )

    res = run_bass_kernel_spmd(nc, in_maps, core_ids=list(range(N_CORES)))
    if _results_hook is not None:
        _results_hook(res)

    out = np.empty((B, S, E), dtype=np.float32)
    for c in range(N_CORES):
        b, q4 = c // 4, c % 4
        out[b, :, 256 * q4 : 256 * (q4 + 1)] = res.results[c]["outT"].T
    return out



# revision 74
# speedup vs baseline: 1.0932x; 1.0137x over previous
"""Multi-head attention kernel for Trainium2, SPMD over 8 NeuronCores.

Problem: B=2, S=2048, E=1024, H=16 heads, Dh=64.
  q = per-head q_in @ Wq.T (Wq shared across heads), same for k, v
  attn = softmax(q k^T / 8); ctx = attn @ v; out = concat(ctx) @ Wo.T + bo

Sharding: core c handles batch b=c//4 and heads 4*(c%4)..4*(c%4)+3
(head-parallel attention).  The out projection is sharded by e_out columns
(each core receives 256 rows of Wo, host-sliced), with an AllGather of the
per-head context over the 4 cores of each batch group in between.

All matmuls run in bf16 with fp32 PSUM accumulation; softmax statistics
(row sums / reciprocals) stay fp32.

Layout tricks (avoid transposing activations for the V path):
  scores^T = kin @ (A @ qin^T)     with A = Wq^T Wk (projection fused)
  ctx^T    = Wv @ (vin^T @ P^T)    (vin used in natural layout)
  rowsum   = extra ones-column on vin (rides the PE contraction for free)
"""

import contextlib
import sys

sys.path.insert(0, "/opt/trn_rl_repo")

import numpy as np

import concourse.bass as bass
import concourse.masks as masks
import concourse.tile as tile
from concourse import bacc, mybir
from concourse.bass_utils import run_bass_kernel_spmd

B, S, E, H, Dh = 2, 2048, 1024, 16, 64
N_CORES = 8
HPC = 4          # heads per core
NK = S // 128    # 16 key chunks
EOUT = E // 4    # e_out columns per core

F32 = mybir.dt.float32
BF16 = mybir.dt.bfloat16

_CACHE = {}


def _declare_io(nc):
    io = {}
    io["qin"] = nc.dram_tensor("qin", [S, HPC * Dh], F32, kind="ExternalInput").ap()
    io["kin"] = nc.dram_tensor("kin", [S, HPC * Dh], F32, kind="ExternalInput").ap()
    io["vin"] = nc.dram_tensor("vin", [S, HPC * Dh], F32, kind="ExternalInput").ap()
    io["wq"] = nc.dram_tensor("wq", [Dh, Dh], F32, kind="ExternalInput").ap()
    io["wk"] = nc.dram_tensor("wk", [Dh, Dh], F32, kind="ExternalInput").ap()
    io["wv"] = nc.dram_tensor("wv", [Dh, Dh], F32, kind="ExternalInput").ap()
    io["wo_s"] = nc.dram_tensor("wo_s", [EOUT, E], F32, kind="ExternalInput").ap()
    io["bo_s"] = nc.dram_tensor("bo_s", [2, 128], F32, kind="ExternalInput").ap()
    io["outT"] = nc.dram_tensor("outT", [EOUT, S], F32, kind="ExternalOutput").ap()
    return io


def _body(nc, tc, es, io, it, collective=True):
    """One full MHA iteration. `it` only namespaces pool names."""

    def pool(name, bufs, space="SBUF"):
        return es.enter_context(
            tc.tile_pool(name=f"{name}_{it}", bufs=bufs, space=space)
        )

    qin, kin, vin = io["qin"], io["kin"], io["vin"]
    wq, wk, wv, wo_s, bo_s, outT = (
        io["wq"], io["wk"], io["wv"], io["wo_s"], io["bo_s"], io["outT"],
    )

    stage = pool("stage", 2)          # fp32/bf16 staging for casts
    persist = pool("persist", 1)      # long-lived bf16 tensors
    psum_big = pool("psum_big", 2, space="PSUM")    # [128,1024] = 2 banks x2
    psum_acc = pool("psum_acc", 1, space="PSUM")    # [*, 2048]  = 4 banks x1
    upool = pool("upool", 2)
    ppool = pool("ppool", 6)
    npool1 = pool("npool1", 1)        # rsr / rs_b (rs gets 2 bufs below)
    npool2 = pool("npool2", 2)        # w2n / ctxT
    dram = pool("dram", 1, space="DRAM")

    # identity for PE transposes
    ident = persist.tile([128, 128], F32, tag="ident")
    masks.make_identity(nc, ident[:])

    # ---------------- tiny weight prep ----------------
    wq_sb = persist.tile([Dh, Dh], F32, tag="wq_sb")
    nc.sync.dma_start(out=wq_sb[:], in_=wq[:, :])
    wk_sb = persist.tile([Dh, Dh], F32, tag="wk_sb")
    nc.sync.dma_start(out=wk_sb[:], in_=wk[:, :])
    wq_bf = persist.tile([Dh, Dh], BF16, tag="wq_bf")
    nc.vector.tensor_copy(wq_bf[:], wq_sb[:])
    wk_bf = persist.tile([Dh, Dh], BF16, tag="wk_bf")
    nc.vector.tensor_copy(wk_bf[:], wk_sb[:])

    # A = Wq^T @ Wk   [64,64]
    a_ps = psum_big.tile([Dh, Dh], F32, tag="big")
    nc.tensor.matmul(a_ps[:], wq_bf[:], wk_bf[:], start=True, stop=True)
    a_bf = persist.tile([Dh, Dh], BF16, tag="a_bf")
    nc.vector.tensor_copy(a_bf[:], a_ps[:])

    # WvT = Wv^T via small strided DMA from DRAM (64x64, one-time)
    wvT_sb = persist.tile([Dh, Dh], F32, tag="wvT_sb")
    nc.sync.dma_start(out=wvT_sb[:], in_=wv.rearrange("a b -> b a"))
    wvT_bf = persist.tile([Dh, Dh], BF16, tag="wvT_bf")
    nc.vector.tensor_copy(wvT_bf[:], wvT_sb[:])

    # ---------------- activations: load + PE block-transpose ----------------
    # qT/kT packs: [128, 2048] bf16; pack g holds heads 2g (rows 0-63), 2g+1 (64-127)
    qT = [persist.tile([128, S], BF16, tag=f"qT{g}", name=f"qT{g}") for g in range(2)]
    kT = [persist.tile([128, S], BF16, tag=f"kT{g}", name=f"kT{g}") for g in range(2)]
    # per-head base-partition-0 views; odd heads are DMA-copied after transpose
    hsplit = {}
    for hh in ("q", "k"):
        for j in (1, 3):
            hsplit[(hh, j)] = persist.tile(
                [Dh, S], BF16, tag=f"hsplit_{hh}{j}", name=f"hsplit_{it}_{hh}{j}"
            )
    qTh = [qT[0][0:Dh, :], hsplit[("q", 1)][:], qT[1][0:Dh, :], hsplit[("q", 3)][:]]
    kTh = [kT[0][0:Dh, :], hsplit[("k", 1)][:], kT[1][0:Dh, :], hsplit[("k", 3)][:]]

    # vin_ones: [128, NK, HPC, 65] bf16 (col 64 = 1.0 rides the contraction)
    vin_ones = persist.tile([128, NK, HPC, Dh + 1], BF16, tag="vin_ones")
    nc.vector.memset(vin_ones[:, :, :, Dh : Dh + 1], 1.0)

    qk_stage_tiles = {}

    def emit_stage_qk(src_ap, packs, hh, g, c0=0, c1=NK, hsplit_dma=True):
        """Load chunks [c0,c1) of one [2048,128] half, PE-transpose into the
        pack, and (once the pack is complete) split off the odd head.
        Chunk-ranged so k staging can be split around the first scores."""
        key = (hh, g)
        if key not in qk_stage_tiles:
            # k g=0 is filled by two chunk-ranged calls; pin it to its own
            # buffer so the pool can't recycle it between the calls
            tag, bufs = ("kst0", 1) if key == ("k", 0) else ("astage", 2)
            qk_stage_tiles[key] = stage.tile(
                [128, NK, 128], F32, tag=tag, bufs=bufs, name=f"st_{it}_{hh}{g}"
            )
        st = qk_stage_tiles[key]
        src_r = src_ap[:, 128 * g : 128 * (g + 1)].rearrange("(c p) d -> p c d", p=128)
        for cb in range(c0, c1, 4):  # 4-chunk loads so transposes start early
            ce = min(cb + 4, c1)
            nc.sync.dma_start(out=st[:, cb:ce, :], in_=src_r[:, cb:ce, :])
        for cq in range(c0 // 4, c1 // 4):  # 4 transposes batched per psum bank
            t_ps = psum_big.tile([128, 512], F32, tag="big", name=f"tp_{it}_{hh}{g}_{cq}")
            for ci in range(4):
                c = 4 * cq + ci
                nc.tensor.transpose(
                    t_ps[:, 128 * ci : 128 * (ci + 1)], st[:, c, :], ident[:]
                )
            nc.vector.tensor_copy(packs[g][:, 512 * cq : 512 * (cq + 1)], t_ps[:])
        if hsplit_dma:
            nc.sync.dma_start(
                out=hsplit[(hh, 2 * g + 1)][:], in_=packs[g][Dh : 2 * Dh, :]
            )

    def emit_stage_v(g):
        st = stage.tile([128, NK, 128], F32, tag="astage", name=f"stv_{it}_{g}")
        nc.sync.dma_start(
            out=st[:],
            in_=vin[:, 128 * g : 128 * (g + 1)].rearrange("(c p) d -> p c d", p=128),
        )
        # DVE for both casts: ACT must stay clear for exp (in-order queue:
        # anything enqueued before exp delays it)
        eng = nc.vector.tensor_copy
        eng(
            vin_ones[:, :, 2 * g : 2 * g + 2, 0:Dh],
            st[:].rearrange("p c (jj d) -> p c jj d", jj=2),
        )

    # k chunks 0-7 first (scores m=0.. need them before anything else in the
    # in-order PE queue), then the full q pack (u needs all 16 chunks), then
    # v.  k chunks 8-15 are staged inside head 0 (mid hook at m=2).
    emit_stage_qk(kin, kT, "k", 0, 0, NK // 2, hsplit_dma=False)
    emit_stage_qk(qin, qT, "q", 0)

    def emit_stage_k0b():
        emit_stage_qk(kin, kT, "k", 0, NK // 2, NK)

    def emit_stage_g1():
        emit_stage_qk(qin, qT, "q", 1)
        emit_stage_qk(kin, kT, "k", 1)

    # single [128, c8, EOUT] tile: the transpose evacuation then needs one
    # strided DVE copy per 4-chunk batch instead of four narrow ones
    woT = persist.tile([128, 8, EOUT], BF16, tag="woT", name="woT")
    bo_sb = persist.tile([128, 2], F32, tag="bo_sb2")

    def emit_wot(r):
        if r == 0:
            for h in range(2):
                nc.sync.dma_start(
                    out=bo_sb[:, h : h + 1],
                    in_=bo_s[h, :].rearrange("(p one) -> p one", one=1),
                )
        w_st = stage.tile([128, E], F32, tag="wostage", bufs=1, name=f"wst_{it}_{r}")
        nc.sync.dma_start(out=w_st[:], in_=wo_s[128 * r : 128 * (r + 1), :])
        for q8 in range(2):  # 4 transposes batched through one psum bank
            t_ps = psum_big.tile([128, 512], F32, tag="big", name=f"wtp_{it}_{r}_{q8}")
            for ci in range(4):
                c8 = 4 * q8 + ci
                nc.tensor.transpose(
                    t_ps[:, 128 * ci : 128 * (ci + 1)], w_st[:, 128 * c8 : 128 * (c8 + 1)], ident[:]
                )
            nc.vector.tensor_copy(
                woT[:, 4 * q8 : 4 * (q8 + 1), 128 * r : 128 * (r + 1)],
                t_ps[:].rearrange("p (c x) -> p c x", c=4),
            )

    # ---------------- attention per head ----------------
    in_cc = dram.tile([2 * Dh, S], BF16)  # heads 0,1 (AG round 0)
    in_cc2h = [
        dram.tile([2 * Dh, S // 2], BF16, name=f"incc2_{it}_{h}", tag=f"incc2{h}")
        for h in range(2)
    ]  # heads 2,3 staged per q-half, contiguous for the split AG
    ag_outs = [
        dram.tile(
            [512, S], BF16,
            addr_space="Local",
            name=f"agout_{it}_{w}", tag=f"agout{w}",
        )
        for w in range(2)
    ]
    ag2h = [
        dram.tile([512, S // 2], BF16, addr_space="Local",
                  name=f"ag2h_{it}_{h}", tag=f"ag2h{h}")
        for h in range(2)
    ]

    def emit_u(j):
        # two psum tiles (2 matmul halves + 1 wide evac each) instead of
        # four: halves the psum rotations and DVE ops on the critical mid
        u_bf = upool.tile([Dh, S], BF16, tag="u", name=f"u_{it}_{j}")
        for t in range(2):
            u_ps = psum_big.tile([Dh, 1024], F32, tag="big", name=f"ups_{it}_{j}_{t}")
            for v in range(2):
                nc.tensor.matmul(
                    u_ps[:, 512 * v : 512 * (v + 1)],
                    a_bf[:],
                    qTh[j][:, 1024 * t + 512 * v : 1024 * t + 512 * (v + 1)],
                    start=True, stop=True,
                )
            nc.vector.tensor_copy(u_bf[:, 1024 * t : 1024 * (t + 1)], u_ps[:])
        return u_bf

    W2_LOOKAHEAD = 3  # score/exp chunk-halves emitted ahead of their W2

    def emit_scores_w2(j, u_bf, mid_emit=None, post_first_exp=None):
        """scores -> exp -> W2 accumulation, then eager psum evacuation.
        W2 emission trails the scores/exp stream by W2_LOOKAHEAD halves so
        the in-order PE queue never head-of-line blocks on psum_acc reuse
        (its evacuation overlaps the next head's first scores).  `mid_emit`
        is a dict {m: callback} fired before chunk m.  Returns (w2_sb, rs)."""
        w2_ps = psum_acc.tile([Dh + 1, S], F32, tag="acc", name=f"w2ps_{it}_{j}")
        pend = []

        def emit_w2(mm, qq, pb):
            for u in range(2):
                nc.tensor.matmul(
                    w2_ps[:, 1024 * qq + 512 * u : 1024 * qq + 512 * (u + 1)],
                    vin_ones[:, mm, j, :],
                    pb[:, 512 * u : 512 * (u + 1)],
                    start=(mm == 0), stop=(mm == NK - 1),
                )

        for m in range(NK):
            if mid_emit is not None and m in mid_emit:
                mid_emit[m]()
            kslice = kTh[j][:, 128 * m : 128 * (m + 1)]
            for qh in range(2):
                sc_ps = psum_big.tile([128, 1024], F32, tag="big", name=f"scps_{it}_{j}_{m}_{qh}")
                for u in range(2):
                    nc.tensor.matmul(
                        sc_ps[:, 512 * u : 512 * (u + 1)],
                        kslice,
                        u_bf[:, 1024 * qh + 512 * u : 1024 * qh + 512 * (u + 1)],
                        start=True, stop=True,
                    )
                p_bf = ppool.tile([128, 1024], BF16, tag="p", name=f"p_{it}_{j}_{m}_{qh}")
                nc.scalar.activation(
                    p_bf[:], sc_ps[:], mybir.ActivationFunctionType.Exp, scale=0.125
                )
                if m == 0 and qh == 0 and post_first_exp is not None:
                    # previous head's rs copies slot in here: ACT would
                    # otherwise idle waiting for this head's next scores
                    post_first_exp()
                pend.append((m, qh, p_bf))
                if len(pend) > W2_LOOKAHEAD:
                    emit_w2(*pend.pop(0))
        while pend:
            emit_w2(*pend.pop(0))
        w2_sb = []
        for qh in range(2):
            w2h = npool2.tile([Dh, S // 2], F32, tag="w2sb", bufs=4, name=f"w2sb_{it}_{j}_{qh}")
            nc.vector.tensor_copy(w2h[:], w2_ps[0:Dh, 1024 * qh : 1024 * (qh + 1)])
            w2_sb.append(w2h[:])

        def emit_rs():
            rs = []
            for qh in range(2):
                rsh = npool1.tile([1, S // 2], F32, tag="rs", bufs=4, name=f"rs_{it}_{j}_{qh}")
                nc.scalar.copy(rsh[:], w2_ps[Dh : Dh + 1, 1024 * qh : 1024 * (qh + 1)])
                rs.append(rsh[:])
            return rs

        return w2_sb, emit_rs

    def emit_ctx_prep(j, qh, rsh):
            rsr = npool1.tile([1, S // 2], F32, tag="rsr", bufs=2, name=f"rsr_{it}_{j}_{qh}")
            nc.vector.reciprocal_approx_fast(out=rsr[:], in_=rsh)
            rs_b = npool1.tile([Dh, S // 2], F32, tag="rs_b", bufs=2, name=f"rsb_{it}_{j}_{qh}")
            nc.gpsimd.partition_broadcast(rs_b[:], rsr[:])
            return rs_b

    def emit_ctx_finish(j, qh, w2h, rs_b):
            w2n_bf = npool2.tile([Dh, S // 2], BF16, tag="w2n", bufs=2, name=f"w2n_{it}_{j}_{qh}")
            nc.vector.tensor_mul(w2n_bf[:], w2h, rs_b[:])
            ctxT_bf = npool2.tile([Dh, S // 2], BF16, tag="ctxT", bufs=2, name=f"ctxT_{it}_{j}_{qh}")
            for t in range(2):
                c_ps = psum_big.tile([Dh, 512], F32, tag="big", name=f"cps_{it}_{j}_{qh}_{t}")
                nc.tensor.matmul(
                    c_ps[:], wvT_bf[:], w2n_bf[:, 512 * t : 512 * (t + 1)],
                    start=True, stop=True,
                )
                nc.vector.tensor_copy(ctxT_bf[:, 512 * t : 512 * (t + 1)], c_ps[:])
            if j < 2:
                nc.sync.dma_start(
                    out=in_cc[Dh * j : Dh * (j + 1), 1024 * qh : 1024 * (qh + 1)],
                    in_=ctxT_bf[:],
                )
            elif qh == 0:
                nc.sync.dma_start(
                    out=in_cc2h[qh][Dh * (j - 2) : Dh * (j - 1), :],
                    in_=ctxT_bf[:],
                )
            else:
                # half 1 goes to the contiguous quarter tiles feeding the
                # final quarter-granular AllGathers
                for t in range(2):
                    nc.sync.dma_start(
                        out=in_cc2q[t][Dh * (j - 2) : Dh * (j - 1), :],
                        in_=ctxT_bf[:, 512 * t : 512 * (t + 1)],
                    )

    def emit_ctx_half(j, qh, w2h, rsh):
        emit_ctx_finish(j, qh, w2h, emit_ctx_prep(j, qh, rsh))

    def emit_ctx(j, w2_sb, rs, after_half=None):
        """normalize + ctx matmuls + staging DMA (deferred one head),
        pipelined in q-halves to keep the serial chain short."""
        for qh in range(2):
            emit_ctx_half(j, qh, w2_sb[qh], rs[qh])
            if after_half is not None:
                after_half(qh)

    # software-pipelined head loop: head j's normalize/ctx is emitted after
    # head j+1's U projection so the in-order PE queue never head-of-line
    # blocks on the (DVE/GPSIMD) normalize chain.  The out projection is
    # split in two rounds around a split AllGather so most of it overlaps
    # the attention phase.
    agch = pool("agch", 1)
    # consolidated spread targets: one tile per AG event -> one spread DMA
    # instead of four (each DMA pays ~1.7us of issue+init latency)
    cch_ev = agch.tile([128, 4, S], BF16, tag="agev", name="agev")
    cch_od = [
        agch.tile([128, 4, S // 2], BF16, tag=f"agod{h}", name=f"agod{h}")
        for h in range(2)
    ]
    opool = pool("opool", 2)
    o_acc = [opool.tile([128, S], F32, tag=f"oacc{h}", bufs=1, name=f"oacc{h}") for h in range(2)]

    def emit_ag(which):
        """AllGather heads (0,1) [which=0] or (2,3) [which=1] of this batch."""
        if collective:
            nc.gpsimd.collective_compute(
                "AllGather",
                mybir.AluOpType.bypass,
                replica_groups=[[0, 1, 2, 3], [4, 5, 6, 7]],
                ins=[in_cc[:, :].opt()],
                outs=[ag_outs[which].opt()],
            )
        else:
            # sim stand-in: a light dep edge; real AG runs on TOPSP silicon
            nc.sync.dma_start(out=ag_outs[which][0:128, :], in_=in_cc[:, :])
        # chunk c8 = heads {2c8, 2c8+1}; AG round `which` supplies parity-
        # matching chunks: ag_outs[w] slab r = heads {4r+2w, 4r+2w+1} = chunk 2r+w
        assert which == 0
        nc.sync.dma_start(
            out=cch_ev[:], in_=ag_outs[which].rearrange("(r p) q -> p r q", p=128)
        )

    def emit_ag2(h):
        """AllGather heads (2,3), q-column half h only, so the tail pipelines."""
        if collective:
            nc.gpsimd.collective_compute(
                "AllGather",
                mybir.AluOpType.bypass,
                replica_groups=[[0, 1, 2, 3], [4, 5, 6, 7]],
                ins=[in_cc2h[h][:, :].opt()],
                outs=[ag2h[h].opt()],
            )
        else:
            nc.sync.dma_start(out=ag2h[h][0:128, :], in_=in_cc2h[h][:, :])
        nc.sync.dma_start(
            out=cch_od[h][:], in_=ag2h[h].rearrange("(r p) q -> p r q", p=128)
        )

    # contiguous quarter staging for the very last AG (collective inputs
    # must be contiguous, so column views of in_cc2h can't be used)
    in_cc2q = [
        dram.tile([2 * Dh, 512], BF16, name=f"incc2q_{it}_{t}", tag=f"incc2q{t}")
        for t in range(2)
    ]
    ag2q = [
        dram.tile([512, 512], BF16, addr_space="Local",
                  name=f"ag2q_{it}_{t}", tag=f"ag2q{t}")
        for t in range(2)
    ]

    def emit_ag2q(t):
        """AllGather heads (2,3), q-column quarter t of half 1."""
        if collective:
            nc.gpsimd.collective_compute(
                "AllGather",
                mybir.AluOpType.bypass,
                replica_groups=[[0, 1, 2, 3], [4, 5, 6, 7]],
                ins=[in_cc2q[t][:, :].opt()],
                outs=[ag2q[t].opt()],
            )
        else:
            nc.scalar.dma_start(out=ag2q[t][0:128, :], in_=in_cc2q[t][:, :])
        # spread issued from the (idle-at-tail) Pool queue: cheap issue slot
        # and no head-of-line blocking behind SP's earlier DMAs
        nc.scalar.dma_start(
            out=cch_od[1][:, :, 512 * t : 512 * (t + 1)],
            in_=ag2q[t].rearrange("(r p) q -> p r q", p=128),
        )

    def emit_oproj_group(round_, sh, h, acc_eng=None):
                o_ps = psum_big.tile([128, 1024], F32, tag="big", name=f"ops_{it}_{round_}_{h}_{sh}")
                for i, r in enumerate(range(4)):
                    c8 = 2 * r + round_
                    for u in range(2):
                        rhs = (
                            cch_ev[:, r, 1024 * sh + 512 * u : 1024 * sh + 512 * (u + 1)]
                            if round_ == 0
                            else cch_od[sh][:, r, 512 * u : 512 * (u + 1)]
                        )
                        nc.tensor.matmul(
                            o_ps[:, 512 * u : 512 * (u + 1)],
                            woT[:, c8, 128 * h : 128 * (h + 1)],
                            rhs,
                            start=(i == 0), stop=(i == 3),
                        )
                if round_ == 0:
                    (acc_eng or nc.vector.tensor_copy)(
                        o_acc[h][:, 1024 * sh : 1024 * (sh + 1)], o_ps[:]
                    )
                else:
                    o_sb = opool.tile([128, 1024], F32, tag="osb", name=f"osb_{it}_{h}_{sh}")
                    eng = nc.vector
                    eng.scalar_tensor_tensor(
                        o_sb[:], o_ps[:], bo_sb[:, h : h + 1],
                        o_acc[h][:, 1024 * sh : 1024 * (sh + 1)],
                        mybir.AluOpType.add, mybir.AluOpType.add,
                    )
                    nc.sync.dma_start(
                        out=outT[128 * h : 128 * (h + 1), 1024 * sh : 1024 * (sh + 1)],
                        in_=o_sb[:],
                    )

    def emit_oproj(round_):
        """Accumulate 4 chunks (parity `round_`) into o_acc (round 0) or
        finish with bias into outT (round 1)."""
        for sh in range(2):
            for h in range(2):
                emit_oproj_group(round_, sh, h)

    def emit_last_head(u_bf, prev_ctx, prev_rs):
        """Head 3 with q-half-outer loops: half 0's normalize/ctx/AG overlap
        half 1's attention, shrinking the serial tail."""
        j = HPC - 1
        w2_ps = psum_acc.tile([Dh + 1, S], F32, tag="acc", name=f"w2ps_{it}_last")
        halves = {}

        def attn_span(s0, s1, mids=None, post_first_exp=None, tail_emits=None):
            """Attention over 512-col q slices [s0, s1) of this head."""
            pend = []
            ns = s1 - s0

            def emit_w2(mm, pb):
                for u in range(ns):
                    nc.tensor.matmul(
                        w2_ps[:, 512 * (s0 + u) : 512 * (s0 + u + 1)],
                        vin_ones[:, mm, j, :],
                        pb[:, 512 * u : 512 * (u + 1)],
                        start=(mm == 0), stop=(mm == NK - 1),
                    )

            for m in range(NK):
                if mids is not None and m in mids:
                    mids[m]()
                kslice = kTh[j][:, 128 * m : 128 * (m + 1)]
                sc_ps = psum_big.tile([128, 512 * ns], F32, tag="big", name=f"scpsL_{s0}_{m}")
                for u in range(ns):
                    nc.tensor.matmul(
                        sc_ps[:, 512 * u : 512 * (u + 1)],
                        kslice,
                        u_bf[:, 512 * (s0 + u) : 512 * (s0 + u + 1)],
                        start=True, stop=True,
                    )
                p_bf = ppool.tile([128, 512 * ns], BF16, tag="p", name=f"pL_{s0}_{m}")
                nc.scalar.activation(
                    p_bf[:], sc_ps[:], mybir.ActivationFunctionType.Exp, scale=0.125
                )
                if m == 0 and post_first_exp is not None:
                    post_first_exp()
                pend.append((m, p_bf))
                if len(pend) > W2_LOOKAHEAD:
                    emit_w2(*pend.pop(0))
            while pend:
                emit_w2(*pend.pop(0))
                # interleaved tail work keeps PE hot (full p-state) through
                # the exp-bound last chunks
                if tail_emits:
                    tail_emits.pop(0)()

        def evac_half(qh):
            w2h = npool2.tile([Dh, S // 2], F32, tag="w2sb", bufs=4, name=f"w2sbL_{qh}")
            nc.vector.tensor_copy(w2h[:], w2_ps[0:Dh, 1024 * qh : 1024 * (qh + 1)])
            rsh = npool1.tile([1, S // 2], F32, tag="rs", bufs=4, name=f"rsL_{qh}")
            nc.scalar.copy(rsh[:], w2_ps[Dh : Dh + 1, 1024 * qh : 1024 * (qh + 1)])
            halves[qh] = (w2h[:], rsh[:])

        def tail_quarter_norm(t):
            """evac + normalize of q-column quarter t of half 1."""
            w2q = npool2.tile([Dh, 512], F32, tag="w2sb", bufs=4, name=f"w2qL_{t}")
            nc.vector.tensor_copy(w2q[:], w2_ps[0:Dh, 1024 + 512 * t : 1024 + 512 * (t + 1)])
            rsq = npool1.tile([1, 512], F32, tag="rs", bufs=4, name=f"rsqL_{t}")
            nc.scalar.copy(rsq[:], w2_ps[Dh : Dh + 1, 1024 + 512 * t : 1024 + 512 * (t + 1)])
            rsrq = npool1.tile([1, 512], F32, tag="rsr", bufs=2, name=f"rsrqL_{t}")
            nc.vector.reciprocal_approx_fast(out=rsrq[:], in_=rsq[:])
            rsbq = npool1.tile([Dh, 512], F32, tag="rs_b", bufs=2, name=f"rsbqL_{t}")
            nc.gpsimd.partition_broadcast(rsbq[:], rsrq[:])
            w2nq = npool2.tile([Dh, 512], BF16, tag="w2n", bufs=2, name=f"w2nqL_{t}")
            nc.vector.tensor_mul(w2nq[:], w2q[:], rsbq[:])
            return w2nq

        def tail_quarter_ctx(t, w2nq):
            c_ps = psum_big.tile([Dh, 512], F32, tag="big", name=f"cpsqL_{t}")
            nc.tensor.matmul(c_ps[:], wvT_bf[:], w2nq[:], start=True, stop=True)
            ctxq = npool2.tile([Dh, 512], BF16, tag="ctxT", bufs=2, name=f"ctxqL_{t}")
            nc.vector.tensor_copy(ctxq[:], c_ps[:])
            nc.scalar.dma_start(out=in_cc2q[t][Dh : 2 * Dh, :], in_=ctxq[:])
            emit_ag2q(t)

        def tail_quarter_oproj(t):
            for h in range(2):
                o_ps = psum_big.tile([128, 512], F32, tag="big", name=f"opsq_{t}_{h}")
                for r in range(4):
                    nc.tensor.matmul(
                        o_ps[:],
                        woT[:, 2 * r + 1, 128 * h : 128 * (h + 1)],
                        cch_od[1][:, r, 512 * t : 512 * (t + 1)],
                        start=(r == 0), stop=(r == 3),
                    )
                o_sb = opool.tile([128, 512], F32, tag="osb", name=f"osbq_{t}_{h}")
                nc.vector.scalar_tensor_tensor(
                    o_sb[:], o_ps[:], bo_sb[:, h : h + 1],
                    o_acc[h][:, 1024 + 512 * t : 1024 + 512 * (t + 1)],
                    mybir.AluOpType.add, mybir.AluOpType.add,
                )
                nc.scalar.dma_start(
                    out=outT[128 * h : 128 * (h + 1), 1024 + 512 * t : 1024 + 512 * (t + 1)],
                    in_=o_sb[:],
                )

        attn_span(0, 2, mids={4: prev_ctx}, post_first_exp=prev_rs)
        evac_half(0)
        # half 1: spread oproj round 0 + half-0 ctx/AG across early chunks so
        # neither PE nor ACT sees a long detour at one point
        attn_span(2, 4, mids={
            1: lambda: [emit_oproj_group(0, 0, h) for h in range(2)],
            3: lambda: (emit_ctx_half(j, 0, *halves[0]), emit_ag2(0)),
            6: lambda: [emit_oproj_group(0, 1, h) for h in range(2)],
        })
        # quartered tail: launch each quarter's AG as early as possible and
        # interleave the long-ready oproj(1,0,*) groups as PE fill (also
        # keeping its p-state at full clock) while the DMA->AG chains drain
        wn0 = tail_quarter_norm(0)
        emit_oproj_group(1, 0, 0)
        tail_quarter_ctx(0, wn0)
        wn1 = tail_quarter_norm(1)
        emit_oproj_group(1, 0, 1)
        tail_quarter_ctx(1, wn1)
        tail_quarter_oproj(0)
        tail_quarter_oproj(1)

    u_next = [emit_u(0)]
    # v staging emitted after u(0) so its Pool-queue cast doesn't head-of-line
    # block u's psum evacuations (which gate the first scores)
    emit_stage_v(0)
    pending = None      # (j, w2_sb) of the previous head
    rs_store = {}       # j -> rs APs (filled by the deferred emitters)
    rs_emitters = {}    # j -> closure that emits the rs copies
    for j in range(HPC - 1):
        mids = {NK // 2: lambda: u_next.append(emit_u(j + 1))}
        if j == 0:
            mids[2] = emit_stage_k0b
            mids[12] = lambda: emit_wot(0)
        if j == 1:
            # g1 staging spread over head 1 (first needed by u(2) at m=8 /
            # head-2 scores) instead of one big detour inside head 0
            mids[2] = lambda: emit_stage_qk(qin, qT, "q", 1)
            mids[5] = lambda: emit_stage_qk(kin, kT, "k", 1)
            mids[11] = lambda: emit_stage_v(1)
        if j == 2:
            mids[5] = lambda: emit_wot(1)

        def post_fe(jp=j - 1):
            if jp in rs_emitters:
                rs_store[jp] = rs_emitters.pop(jp)()

        u_cur = u_next[-1]
        w2_sb, emit_rs = emit_scores_w2(
            j, u_cur, mid_emit=mids, post_first_exp=post_fe if j > 0 else None
        )
        rs_emitters[j] = emit_rs
        if pending is not None:
            jprev, w2_prev = pending
            emit_ctx(jprev, w2_prev, rs_store[jprev])
            if jprev == 1:
                emit_ag(0)
        pending = (j, w2_sb)
    jprev, w2_prev = pending
    emit_last_head(
        u_next[-1],
        prev_ctx=lambda: emit_ctx(jprev, w2_prev, rs_store[jprev]),
        prev_rs=lambda: rs_store.update({jprev: rs_emitters.pop(jprev)()}),
    )


def _build(repeats=1, collective=True):
    key = (repeats, collective)
    if key in _CACHE:
        return _CACHE[key]
    ndev = N_CORES if collective else 1
    nc = bacc.Bacc("TRN2", target_bir_lowering=False, debug=False, num_devices=ndev)
    io = _declare_io(nc)
    with tile.TileContext(nc) as tc:
        for it in range(repeats):
            with contextlib.ExitStack() as es:
                _body(nc, tc, es, io, it, collective=collective)
    nc.compile()
    _CACHE[key] = nc
    return nc


def kernel(k_in, q_in, v_in, Wq, Wk, Wv, Wo, bo, _repeats=1, _results_hook=None):
    k_in = np.asarray(k_in, dtype=np.float32)
    q_in = np.asarray(q_in, dtype=np.float32)
    v_in = np.asarray(v_in, dtype=np.float32)
    Wq = np.ascontiguousarray(np.asarray(Wq, dtype=np.float32))
    Wk = np.ascontiguousarray(np.asarray(Wk, dtype=np.float32))
    Wv = np.ascontiguousarray(np.asarray(Wv, dtype=np.float32))
    Wo = np.asarray(Wo, dtype=np.float32)
    bo = np.asarray(bo, dtype=np.float32)

    nc = _build(_repeats)

    in_maps = []
    for c in range(N_CORES):
        b, q4 = c // 4, c % 4
        sl = slice(256 * q4, 256 * (q4 + 1))
        in_maps.append(
            {
                "qin": np.ascontiguousarray(q_in[b, :, sl]),
                "kin": np.ascontiguousarray(k_in[b, :, sl]),
                "vin": np.ascontiguousarray(v_in[b, :, sl]),
                "wq": Wq,
                "wk": Wk,
                "wv": Wv,
                "wo_s": np.ascontiguousarray(Wo[sl, :]),
                "bo_s": np.ascontiguousarray(bo[sl].reshape(2, 128)),
            }
        )

    res = run_bass_kernel_spmd(nc, in_maps, core_ids=list(range(N_CORES)))
    if _results_hook is not None:
        _results_hook(res)

    out = np.empty((B, S, E), dtype=np.float32)
    for c in range(N_CORES):
        b, q4 = c // 4, c % 4
        out[b, :, 256 * q4 : 256 * (q4 + 1)] = res.results[c]["outT"].T
    return out



# revision 75
# speedup vs baseline: 1.0941x; 1.0008x over previous
"""Multi-head attention kernel for Trainium2, SPMD over 8 NeuronCores.

Problem: B=2, S=2048, E=1024, H=16 heads, Dh=64.
  q = per-head q_in @ Wq.T (Wq shared across heads), same for k, v
  attn = softmax(q k^T / 8); ctx = attn @ v; out = concat(ctx) @ Wo.T + bo

Sharding: core c handles batch b=c//4 and heads 4*(c%4)..4*(c%4)+3
(head-parallel attention).  The out projection is sharded by e_out columns
(each core receives 256 rows of Wo, host-sliced), with an AllGather of the
per-head context over the 4 cores of each batch group in between.

All matmuls run in bf16 with fp32 PSUM accumulation; softmax statistics
(row sums / reciprocals) stay fp32.

Layout tricks (avoid transposing activations for the V path):
  scores^T = kin @ (A @ qin^T)     with A = Wq^T Wk (projection fused)
  ctx^T    = Wv @ (vin^T @ P^T)    (vin used in natural layout)
  rowsum   = extra ones-column on vin (rides the PE contraction for free)
"""

import contextlib
import sys

sys.path.insert(0, "/opt/trn_rl_repo")

import numpy as np

import concourse.bass as bass
import concourse.masks as masks
import concourse.tile as tile
from concourse import bacc, mybir
from concourse.bass_utils import run_bass_kernel_spmd

B, S, E, H, Dh = 2, 2048, 1024, 16, 64
N_CORES = 8
HPC = 4          # heads per core
NK = S // 128    # 16 key chunks
EOUT = E // 4    # e_out columns per core

F32 = mybir.dt.float32
BF16 = mybir.dt.bfloat16

_CACHE = {}


def _declare_io(nc):
    io = {}
    io["qin"] = nc.dram_tensor("qin", [S, HPC * Dh], F32, kind="ExternalInput").ap()
    io["kin"] = nc.dram_tensor("kin", [S, HPC * Dh], F32, kind="ExternalInput").ap()
    io["vin"] = nc.dram_tensor("vin", [S, HPC * Dh], F32, kind="ExternalInput").ap()
    io["wq"] = nc.dram_tensor("wq", [Dh, Dh], F32, kind="ExternalInput").ap()
    io["wk"] = nc.dram_tensor("wk", [Dh, Dh], F32, kind="ExternalInput").ap()
    io["wv"] = nc.dram_tensor("wv", [Dh, Dh], F32, kind="ExternalInput").ap()
    io["wo_s"] = nc.dram_tensor("wo_s", [EOUT, E], F32, kind="ExternalInput").ap()
    io["bo_s"] = nc.dram_tensor("bo_s", [2, 128], F32, kind="ExternalInput").ap()
    io["outT"] = nc.dram_tensor("outT", [EOUT, S], F32, kind="ExternalOutput").ap()
    return io


def _body(nc, tc, es, io, it, collective=True):
    """One full MHA iteration. `it` only namespaces pool names."""

    def pool(name, bufs, space="SBUF"):
        return es.enter_context(
            tc.tile_pool(name=f"{name}_{it}", bufs=bufs, space=space)
        )

    qin, kin, vin = io["qin"], io["kin"], io["vin"]
    wq, wk, wv, wo_s, bo_s, outT = (
        io["wq"], io["wk"], io["wv"], io["wo_s"], io["bo_s"], io["outT"],
    )

    stage = pool("stage", 2)          # fp32/bf16 staging for casts
    persist = pool("persist", 1)      # long-lived bf16 tensors
    psum_big = pool("psum_big", 2, space="PSUM")    # [128,1024] = 2 banks x2
    psum_acc = pool("psum_acc", 1, space="PSUM")    # [*, 2048]  = 4 banks x1
    upool = pool("upool", 2)
    ppool = pool("ppool", 6)
    npool1 = pool("npool1", 1)        # rsr / rs_b (rs gets 2 bufs below)
    npool2 = pool("npool2", 2)        # w2n / ctxT
    dram = pool("dram", 1, space="DRAM")

    # identity for PE transposes
    ident = persist.tile([128, 128], F32, tag="ident")
    masks.make_identity(nc, ident[:])

    # ---------------- tiny weight prep ----------------
    wq_sb = persist.tile([Dh, Dh], F32, tag="wq_sb")
    nc.sync.dma_start(out=wq_sb[:], in_=wq[:, :])
    wk_sb = persist.tile([Dh, Dh], F32, tag="wk_sb")
    nc.sync.dma_start(out=wk_sb[:], in_=wk[:, :])
    wq_bf = persist.tile([Dh, Dh], BF16, tag="wq_bf")
    nc.vector.tensor_copy(wq_bf[:], wq_sb[:])
    wk_bf = persist.tile([Dh, Dh], BF16, tag="wk_bf")
    nc.vector.tensor_copy(wk_bf[:], wk_sb[:])

    # A = Wq^T @ Wk   [64,64]
    a_ps = psum_big.tile([Dh, Dh], F32, tag="big")
    nc.tensor.matmul(a_ps[:], wq_bf[:], wk_bf[:], start=True, stop=True)
    a_bf = persist.tile([Dh, Dh], BF16, tag="a_bf")
    nc.vector.tensor_copy(a_bf[:], a_ps[:])

    # WvT = Wv^T via small strided DMA from DRAM (64x64, one-time)
    wvT_sb = persist.tile([Dh, Dh], F32, tag="wvT_sb")
    nc.sync.dma_start(out=wvT_sb[:], in_=wv.rearrange("a b -> b a"))
    wvT_bf = persist.tile([Dh, Dh], BF16, tag="wvT_bf")
    nc.vector.tensor_copy(wvT_bf[:], wvT_sb[:])

    # ---------------- activations: load + PE block-transpose ----------------
    # qT/kT packs: [128, 2048] bf16; pack g holds heads 2g (rows 0-63), 2g+1 (64-127)
    qT = [persist.tile([128, S], BF16, tag=f"qT{g}", name=f"qT{g}") for g in range(2)]
    kT = [persist.tile([128, S], BF16, tag=f"kT{g}", name=f"kT{g}") for g in range(2)]
    # per-head base-partition-0 views; odd heads are DMA-copied after transpose
    hsplit = {}
    for hh in ("q", "k"):
        for j in (1, 3):
            hsplit[(hh, j)] = persist.tile(
                [Dh, S], BF16, tag=f"hsplit_{hh}{j}", name=f"hsplit_{it}_{hh}{j}"
            )
    qTh = [qT[0][0:Dh, :], hsplit[("q", 1)][:], qT[1][0:Dh, :], hsplit[("q", 3)][:]]
    kTh = [kT[0][0:Dh, :], hsplit[("k", 1)][:], kT[1][0:Dh, :], hsplit[("k", 3)][:]]

    # vin_ones: [128, NK, HPC, 65] bf16 (col 64 = 1.0 rides the contraction)
    vin_ones = persist.tile([128, NK, HPC, Dh + 1], BF16, tag="vin_ones")
    nc.vector.memset(vin_ones[:, :, :, Dh : Dh + 1], 1.0)

    qk_stage_tiles = {}

    def emit_stage_qk(src_ap, packs, hh, g, c0=0, c1=NK, hsplit_dma=True):
        """Load chunks [c0,c1) of one [2048,128] half, PE-transpose into the
        pack, and (once the pack is complete) split off the odd head.
        Chunk-ranged so k staging can be split around the first scores."""
        key = (hh, g)
        if key not in qk_stage_tiles:
            # k g=0 is filled by two chunk-ranged calls; pin it to its own
            # buffer so the pool can't recycle it between the calls
            tag, bufs = ("kst0", 1) if key == ("k", 0) else ("astage", 2)
            qk_stage_tiles[key] = stage.tile(
                [128, NK, 128], F32, tag=tag, bufs=bufs, name=f"st_{it}_{hh}{g}"
            )
        st = qk_stage_tiles[key]
        src_r = src_ap[:, 128 * g : 128 * (g + 1)].rearrange("(c p) d -> p c d", p=128)
        for cb in range(c0, c1, 4):  # 4-chunk loads so transposes start early
            ce = min(cb + 4, c1)
            nc.sync.dma_start(out=st[:, cb:ce, :], in_=src_r[:, cb:ce, :])
        for cq in range(c0 // 4, c1 // 4):  # 4 transposes batched per psum bank
            t_ps = psum_big.tile([128, 512], F32, tag="big", name=f"tp_{it}_{hh}{g}_{cq}")
            for ci in range(4):
                c = 4 * cq + ci
                nc.tensor.transpose(
                    t_ps[:, 128 * ci : 128 * (ci + 1)], st[:, c, :], ident[:]
                )
            nc.vector.tensor_copy(packs[g][:, 512 * cq : 512 * (cq + 1)], t_ps[:])
        if hsplit_dma:
            nc.sync.dma_start(
                out=hsplit[(hh, 2 * g + 1)][:], in_=packs[g][Dh : 2 * Dh, :]
            )

    def emit_stage_v(g):
        st = stage.tile([128, NK, 128], F32, tag="astage", name=f"stv_{it}_{g}")
        nc.sync.dma_start(
            out=st[:],
            in_=vin[:, 128 * g : 128 * (g + 1)].rearrange("(c p) d -> p c d", p=128),
        )
        # DVE for both casts: ACT must stay clear for exp (in-order queue:
        # anything enqueued before exp delays it)
        eng = nc.vector.tensor_copy
        eng(
            vin_ones[:, :, 2 * g : 2 * g + 2, 0:Dh],
            st[:].rearrange("p c (jj d) -> p c jj d", jj=2),
        )

    # k chunks 0-7 first (scores m=0.. need them before anything else in the
    # in-order PE queue), then the full q pack (u needs all 16 chunks), then
    # v.  k chunks 8-15 are staged inside head 0 (mid hook at m=2).
    emit_stage_qk(kin, kT, "k", 0, 0, NK // 2, hsplit_dma=False)
    emit_stage_qk(qin, qT, "q", 0)

    def emit_stage_k0b():
        emit_stage_qk(kin, kT, "k", 0, NK // 2, NK)

    def emit_stage_g1():
        emit_stage_qk(qin, qT, "q", 1)
        emit_stage_qk(kin, kT, "k", 1)

    # single [128, c8, EOUT] tile: the transpose evacuation then needs one
    # strided DVE copy per 4-chunk batch instead of four narrow ones
    woT = persist.tile([128, 8, EOUT], BF16, tag="woT", name="woT")
    bo_sb = persist.tile([128, 2], F32, tag="bo_sb2")

    def emit_wot(r):
        if r == 0:
            for h in range(2):
                nc.sync.dma_start(
                    out=bo_sb[:, h : h + 1],
                    in_=bo_s[h, :].rearrange("(p one) -> p one", one=1),
                )
        w_st = stage.tile([128, E], F32, tag="wostage", bufs=1, name=f"wst_{it}_{r}")
        nc.sync.dma_start(out=w_st[:], in_=wo_s[128 * r : 128 * (r + 1), :])
        for q8 in range(2):  # 4 transposes batched through one psum bank
            t_ps = psum_big.tile([128, 512], F32, tag="big", name=f"wtp_{it}_{r}_{q8}")
            for ci in range(4):
                c8 = 4 * q8 + ci
                nc.tensor.transpose(
                    t_ps[:, 128 * ci : 128 * (ci + 1)], w_st[:, 128 * c8 : 128 * (c8 + 1)], ident[:]
                )
            nc.vector.tensor_copy(
                woT[:, 4 * q8 : 4 * (q8 + 1), 128 * r : 128 * (r + 1)],
                t_ps[:].rearrange("p (c x) -> p c x", c=4),
            )

    # ---------------- attention per head ----------------
    in_cc = dram.tile([2 * Dh, S], BF16)  # heads 0,1 (AG round 0)
    in_cc2h = [
        dram.tile([2 * Dh, S // 2], BF16, name=f"incc2_{it}_{h}", tag=f"incc2{h}")
        for h in range(2)
    ]  # heads 2,3 staged per q-half, contiguous for the split AG
    ag_outs = [
        dram.tile(
            [512, S], BF16,
            addr_space="Local",
            name=f"agout_{it}_{w}", tag=f"agout{w}",
        )
        for w in range(2)
    ]
    ag2h = [
        dram.tile([512, S // 2], BF16, addr_space="Local",
                  name=f"ag2h_{it}_{h}", tag=f"ag2h{h}")
        for h in range(2)
    ]

    def emit_u(j):
        # two psum tiles (2 matmul halves + 1 wide evac each) instead of
        # four: halves the psum rotations and DVE ops on the critical mid
        u_bf = upool.tile([Dh, S], BF16, tag="u", name=f"u_{it}_{j}")
        for t in range(2):
            u_ps = psum_big.tile([Dh, 1024], F32, tag="big", name=f"ups_{it}_{j}_{t}")
            for v in range(2):
                nc.tensor.matmul(
                    u_ps[:, 512 * v : 512 * (v + 1)],
                    a_bf[:],
                    qTh[j][:, 1024 * t + 512 * v : 1024 * t + 512 * (v + 1)],
                    start=True, stop=True,
                )
            nc.vector.tensor_copy(u_bf[:, 1024 * t : 1024 * (t + 1)], u_ps[:])
        return u_bf

    W2_LOOKAHEAD = 3  # score/exp chunk-halves emitted ahead of their W2

    def emit_scores_w2(j, u_bf, mid_emit=None, post_first_exp=None):
        """scores -> exp -> W2 accumulation, then eager psum evacuation.
        W2 emission trails the scores/exp stream by W2_LOOKAHEAD halves so
        the in-order PE queue never head-of-line blocks on psum_acc reuse
        (its evacuation overlaps the next head's first scores).  `mid_emit`
        is a dict {m: callback} fired before chunk m.  Returns (w2_sb, rs)."""
        w2_ps = psum_acc.tile([Dh + 1, S], F32, tag="acc", name=f"w2ps_{it}_{j}")
        pend = []

        def emit_w2(mm, qq, pb):
            for u in range(2):
                nc.tensor.matmul(
                    w2_ps[:, 1024 * qq + 512 * u : 1024 * qq + 512 * (u + 1)],
                    vin_ones[:, mm, j, :],
                    pb[:, 512 * u : 512 * (u + 1)],
                    start=(mm == 0), stop=(mm == NK - 1),
                )

        for m in range(NK):
            if mid_emit is not None and m in mid_emit:
                mid_emit[m]()
            kslice = kTh[j][:, 128 * m : 128 * (m + 1)]
            for qh in range(2):
                sc_ps = psum_big.tile([128, 1024], F32, tag="big", name=f"scps_{it}_{j}_{m}_{qh}")
                for u in range(2):
                    nc.tensor.matmul(
                        sc_ps[:, 512 * u : 512 * (u + 1)],
                        kslice,
                        u_bf[:, 1024 * qh + 512 * u : 1024 * qh + 512 * (u + 1)],
                        start=True, stop=True,
                    )
                p_bf = ppool.tile([128, 1024], BF16, tag="p", name=f"p_{it}_{j}_{m}_{qh}")
                nc.scalar.activation(
                    p_bf[:], sc_ps[:], mybir.ActivationFunctionType.Exp, scale=0.125
                )
                if m == 0 and qh == 0 and post_first_exp is not None:
                    # previous head's rs copies slot in here: ACT would
                    # otherwise idle waiting for this head's next scores
                    post_first_exp()
                pend.append((m, qh, p_bf))
                if len(pend) > W2_LOOKAHEAD:
                    emit_w2(*pend.pop(0))
        while pend:
            emit_w2(*pend.pop(0))
        w2_sb = []
        for qh in range(2):
            w2h = npool2.tile([Dh, S // 2], F32, tag="w2sb", bufs=4, name=f"w2sb_{it}_{j}_{qh}")
            nc.vector.tensor_copy(w2h[:], w2_ps[0:Dh, 1024 * qh : 1024 * (qh + 1)])
            w2_sb.append(w2h[:])

        def emit_rs():
            rs = []
            for qh in range(2):
                rsh = npool1.tile([1, S // 2], F32, tag="rs", bufs=4, name=f"rs_{it}_{j}_{qh}")
                nc.scalar.copy(rsh[:], w2_ps[Dh : Dh + 1, 1024 * qh : 1024 * (qh + 1)])
                rs.append(rsh[:])
            return rs

        return w2_sb, emit_rs

    def emit_ctx_prep(j, qh, rsh):
            rsr = npool1.tile([1, S // 2], F32, tag="rsr", bufs=2, name=f"rsr_{it}_{j}_{qh}")
            nc.vector.reciprocal_approx_fast(out=rsr[:], in_=rsh)
            rs_b = npool1.tile([Dh, S // 2], F32, tag="rs_b", bufs=2, name=f"rsb_{it}_{j}_{qh}")
            nc.gpsimd.partition_broadcast(rs_b[:], rsr[:])
            return rs_b

    def emit_ctx_finish(j, qh, w2h, rs_b):
            w2n_bf = npool2.tile([Dh, S // 2], BF16, tag="w2n", bufs=2, name=f"w2n_{it}_{j}_{qh}")
            nc.vector.tensor_mul(w2n_bf[:], w2h, rs_b[:])
            ctxT_bf = npool2.tile([Dh, S // 2], BF16, tag="ctxT", bufs=2, name=f"ctxT_{it}_{j}_{qh}")
            c_ps = psum_big.tile([Dh, 1024], F32, tag="big", name=f"cps_{it}_{j}_{qh}")
            for t in range(2):
                nc.tensor.matmul(
                    c_ps[:, 512 * t : 512 * (t + 1)],
                    wvT_bf[:], w2n_bf[:, 512 * t : 512 * (t + 1)],
                    start=True, stop=True,
                )
            nc.vector.tensor_copy(ctxT_bf[:], c_ps[:])
            if j < 2:
                nc.sync.dma_start(
                    out=in_cc[Dh * j : Dh * (j + 1), 1024 * qh : 1024 * (qh + 1)],
                    in_=ctxT_bf[:],
                )
            elif qh == 0:
                nc.sync.dma_start(
                    out=in_cc2h[qh][Dh * (j - 2) : Dh * (j - 1), :],
                    in_=ctxT_bf[:],
                )
            else:
                # half 1 goes to the contiguous quarter tiles feeding the
                # final quarter-granular AllGathers
                for t in range(2):
                    nc.sync.dma_start(
                        out=in_cc2q[t][Dh * (j - 2) : Dh * (j - 1), :],
                        in_=ctxT_bf[:, 512 * t : 512 * (t + 1)],
                    )

    def emit_ctx_half(j, qh, w2h, rsh):
        emit_ctx_finish(j, qh, w2h, emit_ctx_prep(j, qh, rsh))

    def emit_ctx(j, w2_sb, rs, after_half=None):
        """normalize + ctx matmuls + staging DMA (deferred one head),
        pipelined in q-halves to keep the serial chain short."""
        for qh in range(2):
            emit_ctx_half(j, qh, w2_sb[qh], rs[qh])
            if after_half is not None:
                after_half(qh)

    # software-pipelined head loop: head j's normalize/ctx is emitted after
    # head j+1's U projection so the in-order PE queue never head-of-line
    # blocks on the (DVE/GPSIMD) normalize chain.  The out projection is
    # split in two rounds around a split AllGather so most of it overlaps
    # the attention phase.
    agch = pool("agch", 1)
    # consolidated spread targets: one tile per AG event -> one spread DMA
    # instead of four (each DMA pays ~1.7us of issue+init latency)
    cch_ev = agch.tile([128, 4, S], BF16, tag="agev", name="agev")
    cch_od = [
        agch.tile([128, 4, S // 2], BF16, tag=f"agod{h}", name=f"agod{h}")
        for h in range(2)
    ]
    opool = pool("opool", 2)
    o_acc = [opool.tile([128, S], F32, tag=f"oacc{h}", bufs=1, name=f"oacc{h}") for h in range(2)]

    def emit_ag(which):
        """AllGather heads (0,1) [which=0] or (2,3) [which=1] of this batch."""
        if collective:
            nc.gpsimd.collective_compute(
                "AllGather",
                mybir.AluOpType.bypass,
                replica_groups=[[0, 1, 2, 3], [4, 5, 6, 7]],
                ins=[in_cc[:, :].opt()],
                outs=[ag_outs[which].opt()],
            )
        else:
            # sim stand-in: a light dep edge; real AG runs on TOPSP silicon
            nc.sync.dma_start(out=ag_outs[which][0:128, :], in_=in_cc[:, :])
        # chunk c8 = heads {2c8, 2c8+1}; AG round `which` supplies parity-
        # matching chunks: ag_outs[w] slab r = heads {4r+2w, 4r+2w+1} = chunk 2r+w
        assert which == 0
        nc.sync.dma_start(
            out=cch_ev[:], in_=ag_outs[which].rearrange("(r p) q -> p r q", p=128)
        )

    def emit_ag2(h):
        """AllGather heads (2,3), q-column half h only, so the tail pipelines."""
        if collective:
            nc.gpsimd.collective_compute(
                "AllGather",
                mybir.AluOpType.bypass,
                replica_groups=[[0, 1, 2, 3], [4, 5, 6, 7]],
                ins=[in_cc2h[h][:, :].opt()],
                outs=[ag2h[h].opt()],
            )
        else:
            nc.sync.dma_start(out=ag2h[h][0:128, :], in_=in_cc2h[h][:, :])
        nc.sync.dma_start(
            out=cch_od[h][:], in_=ag2h[h].rearrange("(r p) q -> p r q", p=128)
        )

    # contiguous quarter staging for the very last AG (collective inputs
    # must be contiguous, so column views of in_cc2h can't be used)
    in_cc2q = [
        dram.tile([2 * Dh, 512], BF16, name=f"incc2q_{it}_{t}", tag=f"incc2q{t}")
        for t in range(2)
    ]
    ag2q = [
        dram.tile([512, 512], BF16, addr_space="Local",
                  name=f"ag2q_{it}_{t}", tag=f"ag2q{t}")
        for t in range(2)
    ]

    def emit_ag2q(t):
        """AllGather heads (2,3), q-column quarter t of half 1."""
        if collective:
            nc.gpsimd.collective_compute(
                "AllGather",
                mybir.AluOpType.bypass,
                replica_groups=[[0, 1, 2, 3], [4, 5, 6, 7]],
                ins=[in_cc2q[t][:, :].opt()],
                outs=[ag2q[t].opt()],
            )
        else:
            nc.scalar.dma_start(out=ag2q[t][0:128, :], in_=in_cc2q[t][:, :])
        # spread issued from the (idle-at-tail) Pool queue: cheap issue slot
        # and no head-of-line blocking behind SP's earlier DMAs
        nc.scalar.dma_start(
            out=cch_od[1][:, :, 512 * t : 512 * (t + 1)],
            in_=ag2q[t].rearrange("(r p) q -> p r q", p=128),
        )

    def emit_oproj_group(round_, sh, h, acc_eng=None):
                o_ps = psum_big.tile([128, 1024], F32, tag="big", name=f"ops_{it}_{round_}_{h}_{sh}")
                for i, r in enumerate(range(4)):
                    c8 = 2 * r + round_
                    for u in range(2):
                        rhs = (
                            cch_ev[:, r, 1024 * sh + 512 * u : 1024 * sh + 512 * (u + 1)]
                            if round_ == 0
                            else cch_od[sh][:, r, 512 * u : 512 * (u + 1)]
                        )
                        nc.tensor.matmul(
                            o_ps[:, 512 * u : 512 * (u + 1)],
                            woT[:, c8, 128 * h : 128 * (h + 1)],
                            rhs,
                            start=(i == 0), stop=(i == 3),
                        )
                if round_ == 0:
                    (acc_eng or nc.vector.tensor_copy)(
                        o_acc[h][:, 1024 * sh : 1024 * (sh + 1)], o_ps[:]
                    )
                else:
                    o_sb = opool.tile([128, 1024], F32, tag="osb", name=f"osb_{it}_{h}_{sh}")
                    eng = nc.vector
                    eng.scalar_tensor_tensor(
                        o_sb[:], o_ps[:], bo_sb[:, h : h + 1],
                        o_acc[h][:, 1024 * sh : 1024 * (sh + 1)],
                        mybir.AluOpType.add, mybir.AluOpType.add,
                    )
                    nc.sync.dma_start(
                        out=outT[128 * h : 128 * (h + 1), 1024 * sh : 1024 * (sh + 1)],
                        in_=o_sb[:],
                    )

    def emit_oproj(round_):
        """Accumulate 4 chunks (parity `round_`) into o_acc (round 0) or
        finish with bias into outT (round 1)."""
        for sh in range(2):
            for h in range(2):
                emit_oproj_group(round_, sh, h)

    def emit_last_head(u_bf, prev_ctx, prev_rs):
        """Head 3 with q-half-outer loops: half 0's normalize/ctx/AG overlap
        half 1's attention, shrinking the serial tail."""
        j = HPC - 1
        w2_ps = psum_acc.tile([Dh + 1, S], F32, tag="acc", name=f"w2ps_{it}_last")
        halves = {}

        def attn_span(s0, s1, mids=None, post_first_exp=None, tail_emits=None):
            """Attention over 512-col q slices [s0, s1) of this head."""
            pend = []
            ns = s1 - s0

            def emit_w2(mm, pb):
                for u in range(ns):
                    nc.tensor.matmul(
                        w2_ps[:, 512 * (s0 + u) : 512 * (s0 + u + 1)],
                        vin_ones[:, mm, j, :],
                        pb[:, 512 * u : 512 * (u + 1)],
                        start=(mm == 0), stop=(mm == NK - 1),
                    )

            for m in range(NK):
                if mids is not None and m in mids:
                    mids[m]()
                kslice = kTh[j][:, 128 * m : 128 * (m + 1)]
                sc_ps = psum_big.tile([128, 512 * ns], F32, tag="big", name=f"scpsL_{s0}_{m}")
                for u in range(ns):
                    nc.tensor.matmul(
                        sc_ps[:, 512 * u : 512 * (u + 1)],
                        kslice,
                        u_bf[:, 512 * (s0 + u) : 512 * (s0 + u + 1)],
                        start=True, stop=True,
                    )
                p_bf = ppool.tile([128, 512 * ns], BF16, tag="p", name=f"pL_{s0}_{m}")
                nc.scalar.activation(
                    p_bf[:], sc_ps[:], mybir.ActivationFunctionType.Exp, scale=0.125
                )
                if m == 0 and post_first_exp is not None:
                    post_first_exp()
                pend.append((m, p_bf))
                if len(pend) > W2_LOOKAHEAD:
                    emit_w2(*pend.pop(0))
            while pend:
                emit_w2(*pend.pop(0))
                # interleaved tail work keeps PE hot (full p-state) through
                # the exp-bound last chunks
                if tail_emits:
                    tail_emits.pop(0)()

        def evac_half(qh):
            w2h = npool2.tile([Dh, S // 2], F32, tag="w2sb", bufs=4, name=f"w2sbL_{qh}")
            nc.vector.tensor_copy(w2h[:], w2_ps[0:Dh, 1024 * qh : 1024 * (qh + 1)])
            rsh = npool1.tile([1, S // 2], F32, tag="rs", bufs=4, name=f"rsL_{qh}")
            nc.scalar.copy(rsh[:], w2_ps[Dh : Dh + 1, 1024 * qh : 1024 * (qh + 1)])
            halves[qh] = (w2h[:], rsh[:])

        def tail_quarter_norm(t):
            """evac + normalize of q-column quarter t of half 1."""
            w2q = npool2.tile([Dh, 512], F32, tag="w2sb", bufs=4, name=f"w2qL_{t}")
            nc.vector.tensor_copy(w2q[:], w2_ps[0:Dh, 1024 + 512 * t : 1024 + 512 * (t + 1)])
            rsq = npool1.tile([1, 512], F32, tag="rs", bufs=4, name=f"rsqL_{t}")
            nc.scalar.copy(rsq[:], w2_ps[Dh : Dh + 1, 1024 + 512 * t : 1024 + 512 * (t + 1)])
            rsrq = npool1.tile([1, 512], F32, tag="rsr", bufs=2, name=f"rsrqL_{t}")
            nc.vector.reciprocal_approx_fast(out=rsrq[:], in_=rsq[:])
            rsbq = npool1.tile([Dh, 512], F32, tag="rs_b", bufs=2, name=f"rsbqL_{t}")
            nc.gpsimd.partition_broadcast(rsbq[:], rsrq[:])
            w2nq = npool2.tile([Dh, 512], BF16, tag="w2n", bufs=2, name=f"w2nqL_{t}")
            nc.vector.tensor_mul(w2nq[:], w2q[:], rsbq[:])
            return w2nq

        def tail_quarter_ctx(t, w2nq):
            c_ps = psum_big.tile([Dh, 512], F32, tag="big", name=f"cpsqL_{t}")
            nc.tensor.matmul(c_ps[:], wvT_bf[:], w2nq[:], start=True, stop=True)
            ctxq = npool2.tile([Dh, 512], BF16, tag="ctxT", bufs=2, name=f"ctxqL_{t}")
            nc.vector.tensor_copy(ctxq[:], c_ps[:])
            nc.scalar.dma_start(out=in_cc2q[t][Dh : 2 * Dh, :], in_=ctxq[:])
            emit_ag2q(t)

        def tail_quarter_oproj(t):
            for h in range(2):
                o_ps = psum_big.tile([128, 512], F32, tag="big", name=f"opsq_{t}_{h}")
                for r in range(4):
                    nc.tensor.matmul(
                        o_ps[:],
                        woT[:, 2 * r + 1, 128 * h : 128 * (h + 1)],
                        cch_od[1][:, r, 512 * t : 512 * (t + 1)],
                        start=(r == 0), stop=(r == 3),
                    )
                o_sb = opool.tile([128, 512], F32, tag="osb", name=f"osbq_{t}_{h}")
                nc.vector.scalar_tensor_tensor(
                    o_sb[:], o_ps[:], bo_sb[:, h : h + 1],
                    o_acc[h][:, 1024 + 512 * t : 1024 + 512 * (t + 1)],
                    mybir.AluOpType.add, mybir.AluOpType.add,
                )
                nc.scalar.dma_start(
                    out=outT[128 * h : 128 * (h + 1), 1024 + 512 * t : 1024 + 512 * (t + 1)],
                    in_=o_sb[:],
                )

        attn_span(0, 2, mids={4: prev_ctx}, post_first_exp=prev_rs)
        evac_half(0)
        # half 1: spread oproj round 0 + half-0 ctx/AG across early chunks so
        # neither PE nor ACT sees a long detour at one point
        attn_span(2, 4, mids={
            1: lambda: [emit_oproj_group(0, 0, h) for h in range(2)],
            3: lambda: (emit_ctx_half(j, 0, *halves[0]), emit_ag2(0)),
            6: lambda: [emit_oproj_group(0, 1, h) for h in range(2)],
        })
        # quartered tail: launch each quarter's AG as early as possible and
        # interleave the long-ready oproj(1,0,*) groups as PE fill (also
        # keeping its p-state at full clock) while the DMA->AG chains drain
        wn0 = tail_quarter_norm(0)
        emit_oproj_group(1, 0, 0)
        tail_quarter_ctx(0, wn0)
        wn1 = tail_quarter_norm(1)
        emit_oproj_group(1, 0, 1)
        tail_quarter_ctx(1, wn1)
        tail_quarter_oproj(0)
        tail_quarter_oproj(1)

    u_next = [emit_u(0)]
    # v staging emitted after u(0) so its Pool-queue cast doesn't head-of-line
    # block u's psum evacuations (which gate the first scores)
    emit_stage_v(0)
    pending = None      # (j, w2_sb) of the previous head
    rs_store = {}       # j -> rs APs (filled by the deferred emitters)
    rs_emitters = {}    # j -> closure that emits the rs copies
    for j in range(HPC - 1):
        mids = {NK // 2: lambda: u_next.append(emit_u(j + 1))}
        if j == 0:
            mids[2] = emit_stage_k0b
            mids[12] = lambda: emit_wot(0)
        if j == 1:
            # g1 staging spread over head 1 (first needed by u(2) at m=8 /
            # head-2 scores) instead of one big detour inside head 0
            mids[2] = lambda: emit_stage_qk(qin, qT, "q", 1)
            mids[5] = lambda: emit_stage_qk(kin, kT, "k", 1)
            mids[11] = lambda: emit_stage_v(1)
        if j == 2:
            mids[5] = lambda: emit_wot(1)

        def post_fe(jp=j - 1):
            if jp in rs_emitters:
                rs_store[jp] = rs_emitters.pop(jp)()

        u_cur = u_next[-1]
        w2_sb, emit_rs = emit_scores_w2(
            j, u_cur, mid_emit=mids, post_first_exp=post_fe if j > 0 else None
        )
        rs_emitters[j] = emit_rs
        if pending is not None:
            jprev, w2_prev = pending
            emit_ctx(jprev, w2_prev, rs_store[jprev])
            if jprev == 1:
                emit_ag(0)
        pending = (j, w2_sb)
    jprev, w2_prev = pending
    emit_last_head(
        u_next[-1],
        prev_ctx=lambda: emit_ctx(jprev, w2_prev, rs_store[jprev]),
        prev_rs=lambda: rs_store.update({jprev: rs_emitters.pop(jprev)()}),
    )


def _build(repeats=1, collective=True):
    key = (repeats, collective)
    if key in _CACHE:
        return _CACHE[key]
    ndev = N_CORES if collective else 1
    nc = bacc.Bacc("TRN2", target_bir_lowering=False, debug=False, num_devices=ndev)
    io = _declare_io(nc)
    with tile.TileContext(nc) as tc:
        for it in range(repeats):
            with contextlib.ExitStack() as es:
                _body(nc, tc, es, io, it, collective=collective)
    nc.compile()
    _CACHE[key] = nc
    return nc


def kernel(k_in, q_in, v_in, Wq, Wk, Wv, Wo, bo, _repeats=1, _results_hook=None):
    k_in = np.asarray(k_in, dtype=np.float32)
    q_in = np.asarray(q_in, dtype=np.float32)
    v_in = np.asarray(v_in, dtype=np.float32)
    Wq = np.ascontiguousarray(np.asarray(Wq, dtype=np.float32))
    Wk = np.ascontiguousarray(np.asarray(Wk, dtype=np.float32))
    Wv = np.ascontiguousarray(np.asarray(Wv, dtype=np.float32))
    Wo = np.asarray(Wo, dtype=np.float32)
    bo = np.asarray(bo, dtype=np.float32)

    nc = _build(_repeats)

    in_maps = []
    for c in range(N_CORES):
        b, q4 = c // 4, c % 4
        sl = slice(256 * q4, 256 * (q4 + 1))
        in_maps.append(
            {
                "qin": np.ascontiguousarray(q_in[b, :, sl]),
                "kin": np.ascontiguousarray(k_in[b, :, sl]),
                "vin": np.ascontiguousarray(v_in[b, :, sl]),
                "wq": Wq,
                "wk": Wk,
                "wv": Wv,
                "wo_s": np.ascontiguousarray(Wo[sl, :]),
                "bo_s": np.ascontiguousarray(bo[sl].reshape(2, 128)),
            }
        )

    res = run_bass_kernel_spmd(nc, in_maps, core_ids=list(range(N_CORES)))
    if _results_hook is not None:
        _results_hook(res)

    out = np.empty((B, S, E), dtype=np.float32)
    for c in range(N_CORES):
        b, q4 = c // 4, c % 4
        out[b, :, 256 * q4 : 256 * (q4 + 1)] = res.results[c]["outT"].T
    return out

